# revision 1
# baseline (speedup 1.0000x reference)
"""Trainium2 Bass kernel for nn_DR_CML (data-parallel over batch, 8 cores).

Math: xm[b,i,j] = x[b,i]*lm_w[j] + lm_b[j] means every row of `loo` is a
linear function of the scalar s[b,i] = xbar[b] - x[b,i]/xd.  The tiny
H=7 MLPs applied to loo collapse to scalar piecewise-linear functions of
s, and sum_i over the [B,K,xd-1] diff tensor collapses to a quadratic in
y with per-row coefficients.  positive[b,k] is itself a quadratic in
y_k, so 511*positive folds into the same per-row quadratic (rows 0:64
only), with a small col-0 delta for the k=0 base-path override:
    R[p,k] = q2[p]*y^2 + q1[p]*y + q0[p]   (+ pos-fold on lower rows)
    P[c,k] = sum_p F128[p,c] * R[p,k]      (one PE matmul, pair-sum free)
with F128 = [f0 | f0*w0 | f1 | f1*w1] computed on all 128 partitions
(the pair-sum matmul with the full M stationary makes per-partition
values equal across halves).

Key device tricks vs the v1 kernel:
  - C,D row-sums ride the Exp/Tanh activations via accum_out (ACT does
    the reduce; DVE only does em/emm).
  - No s tile: xbar folds into per-partition ACT bias APs computed
    straight from the PSUM pair-sum xbs.
  - psw arrives as 2 rows (2KB) and is partition-broadcast by one PE
    matmul into PSUM; the propensity dot rides a DVE STT accum.
  - One final matmul F128^T @ R128 replaces pair-sum + pos chain.

Layout: x is repacked [2*(B/8), xd/2] = [128, 256]; per-row sums are
halved per partition and pair-summed with one PE matmul against M
(M[p,i]=1 iff i==p or i==p^64).  Each core emits a [4,33] tile of
masked partial sums; the host sums 8 tiles and applies the final
formula.
"""
import math

import numpy as np

B, XD, K, H = 512, 512, 32, 7
NCORES = 8
BL = B // NCORES          # 64 rows per core
HC = XD // 2              # 256 columns after repack
N1 = XD - 1
LN2 = math.log(2.0)
LNG = math.log((XD - 1) / 2.0)   # ge2 bias: exp(-lv + LNG) = 255.5*e^-lv

_prog_cache = {}


def _fold_consts(p):
    """Fold linear_map + MLP weights into scalar-MLP coefficients (f64)."""
    lm_w = p['lm_w'].astype(np.float64)
    lm_b = p['lm_b'].astype(np.float64)
    c = lm_b * (XD - 1) / XD

    def fold(w1, b1):
        u = lm_w @ w1.astype(np.float64)
        v_base = lm_b @ w1.astype(np.float64) + b1.astype(np.float64)
        v_c = c @ w1.astype(np.float64) + b1.astype(np.float64)
        return u, v_base, v_c

    u_mu, vb_mu, vc_mu = fold(p['mu_w1'], p['mu_b1'])
    u_lv, vb_lv, vc_lv = fold(p['lv_w1'], p['lv_b1'])
    u_mun, _, vc_mun = fold(p['mun_w1'], p['mun_b1'])
    u_lvn, _, vc_lvn = fold(p['lvn_w1'], p['lvn_b1'])

    return {
        'u_mu': u_mu, 'vb_mu': vb_mu, 'vc_mu': vc_mu,
        'u_lv': u_lv, 'vb_lv': vb_lv, 'vc_lv': vc_lv,
        'u_mun': u_mun, 'vc_mun': vc_mun,
        'u_lvn': u_lvn, 'vc_lvn': vc_lvn,
        'w2_mu': p['mu_w2'][:, 0].astype(np.float64),
        'w2_lv': p['lv_w2'][:, 0].astype(np.float64),
        'w2_mun': p['mun_w2'][:, 0].astype(np.float64),
        'w2_lvn': p['lvn_w2'][:, 0].astype(np.float64),
        'b2_mu': float(p['mu_b2'][0]), 'b2_lv': float(p['lv_b2'][0]),
        'b2_mun': float(p['mun_b2'][0]), 'b2_lvn': float(p['lvn_b2'][0]),
        'ps_b': float(p['ps_b'][0]),
    }


def _specialize(fc, x):
    """Exact per-call relu pruning over the data's s range (i <= xd-2)."""
    x64 = x.astype(np.float64)
    xbar = x64.mean(1)
    s = xbar[:, None] - x64[:, :N1] / XD
    smin, smax = float(s.min()), float(s.max())

    out = {}
    for name, u_all, v_all, w2_all, b2 in (
            ('mun', fc['u_mun'], fc['vc_mun'], fc['w2_mun'], fc['b2_mun']),
            ('lvn', fc['u_lvn'], fc['vc_lvn'], fc['w2_lvn'], fc['b2_lvn'])):
        alpha, beta = b2, 0.0
        active = []
        for u, v, w2 in zip(u_all, v_all, w2_all):
            if w2 == 0.0:
                continue
            lo = min(u * smin, u * smax) + v
            hi = max(u * smin, u * smax) + v
            if lo >= 0.0:          # linear over the data
                alpha += w2 * v
                beta += w2 * u
            elif hi <= 0.0:        # identically zero over the data
                pass
            else:                  # genuinely piecewise on the data
                active.append((float(abs(w2) * u), float(abs(w2) * v),
                               1.0 if w2 > 0 else -1.0))
        out[name] = (float(alpha), float(beta), active)
    return out


def _const_layout(fc, spec):
    """Column layout of the packed consts tensor + bias-value table.

    M lives in its own bf16 tensor; consts carries bias cols, the
    positive-branch coefficient cols, and a [64,64] f32 identity.
    """
    bias_vals = [0.0, -LN2, LNG, -fc['ps_b'], fc['b2_lv']]
    seen, ordered = set(), []
    for v in bias_vals:
        if v not in seen:
            seen.add(v)
            ordered.append(v)
    nb = len(ordered)
    lay = {
        'bias_vals': ordered,
        'bias0': 0,
        'posa': nb,
        'posc': nb + 1,
        'w2sel': nb + 2,              # 4 cols
        'iden': nb + 6,               # 64 cols
        'width': nb + 6 + 64,
    }
    return lay


def _build_program(fc, spec, lay):
    from contextlib import ExitStack
    import concourse.tile as tile
    from concourse import bacc, mybir
    from concourse.tile import add_dep_helper

    f32 = mybir.dt.float32
    Alu = mybir.AluOpType
    Act = mybir.ActivationFunctionType

    nc = bacc.Bacc("TRN2", target_bir_lowering=False, debug=False,
                   num_devices=NCORES)

    bf16 = mybir.dt.bfloat16
    xt_d = nc.dram_tensor("xt", [2 * BL, HC], f32, kind="ExternalInput").ap()
    yv_d = nc.dram_tensor("yv", [2 * BL, K], f32, kind="ExternalInput").ap()
    pw_d = nc.dram_tensor("pw", [2, HC + 128], bf16,
                          kind="ExternalInput").ap()
    mb_d = nc.dram_tensor("mb", [128, 132], bf16, kind="ExternalInput").ap()
    tc_d = nc.dram_tensor("consts", [128, lay['width']], f32,
                          kind="ExternalInput").ap()
    out_d = nc.dram_tensor("out", [4, K + 1], f32, kind="ExternalOutput").ap()

    bias_idx = {v: lay['bias0'] + i for i, v in enumerate(lay['bias_vals'])}
    a_mun, b_mun, act_mun = spec['mun']
    a_lvn, b_lvn, act_lvn = spec['lvn']
    GHALF = float((XD - 1) / 2.0)   # 255.5

    with tile.TileContext(nc) as tcx, ExitStack() as ctx:
        sb = ctx.enter_context(tcx.tile_pool(name="sb", bufs=1))
        ps = ctx.enter_context(tcx.tile_pool(name="ps", bufs=1, space="PSUM"))

        # ---- DMAs: x alone on sync; consts on PE queue; y+psw on gpsimd
        tx = sb.tile([128, HC], f32, tag="tx")
        nc.sync.dma_start(tx[0:BL, :], xt_d[0:BL, :])
        nc.scalar.dma_start(tx[BL:128, :], xt_d[BL:128, :])
        tm = sb.tile([128, 132], bf16, tag="tm")
        nc.gpsimd.dma_start(tm[:], mb_d)
        tc = sb.tile([128, lay['width']], f32, tag="tc")
        nc.scalar.dma_start(tc[:], tc_d)
        tpw = sb.tile([2, HC + 128], bf16, tag="tpw")
        nc.gpsimd.dma_start(tpw[:], pw_d)
        ty = sb.tile([128, K], f32, tag="ty")
        nc.gpsimd.dma_start(ty[:], yv_d)

        gwarm = sb.tile([1, 1], f32, tag="gwarm")
        nc.gpsimd.tensor_scalar(gwarm[:], nc.const_aps.tensor(0.0, (1, 1)),
                                1.0, None, Alu.add)
        # hoist the ACT table load before any data arrives
        warm = sb.tile([1, 1], f32, tag="warm")
        nc.scalar.activation(warm[:], nc.const_aps.tensor(0.0, (1, 1)),
                             Act.Exp, bias=0.0, scale=1.0)

        def bc(val, p0=0, p1=128):
            j = bias_idx[val]
            return tc[p0:p1, j:j + 1]

        M = tm[:, 0:128]
        iden = tc[0:BL, lay['iden']:lay['iden'] + BL]

        # ================= PE queue =================
        # psw partition-broadcast: [2,256] -> [128,256] PSUM
        pwb = ps.tile([128, HC], f32, tag="pwb")
        pwb_inst = nc.tensor.matmul(pwb[:], tpw[0:2, HC:HC + 128],
                                    tpw[0:2, 0:HC], start=True, stop=True)

        # ================= DVE: xsum + propensity dot =================
        xsum = sb.tile([128, 1], bf16, tag="xsum")
        with nc.allow_low_precision(reason="bf16 pair-sum moving, 0.4% ok"):
            nc.vector.tensor_reduce(xsum[:], tx[:], mybir.AxisListType.X,
                                    Alu.add)

        # stg natively bf16 on DVE: gpsimd's queue is stuffed with DMA
        # issues, and sel_a heads the positive-branch critical chain
        stg = sb.tile([128, 3], bf16, tag="stg")  # [s_raw | treat | pdot]
        with nc.allow_low_precision(reason="bf16 pair-sum moving, 0.4% ok"):
            nc.vector.memset(stg[0:BL, 0:2], 0.0)
            nc.vector.tensor_scalar(stg[BL:128, 0:1],
                                    tx[BL:128, HC - 2:HC - 1],
                                    -1.0 / XD, None, Alu.mult)
            nc.vector.tensor_copy(stg[BL:128, 1:2], tx[BL:128, HC - 1:HC])
        # ---- pair-sum on PE: xbs (full M stationary -> 128 rows)
        # mm1 first on PE so the bias chain (everything downstream)
        # isn't clocked behind the psw broadcast
        xbs = ps.tile([128, 1], f32, tag="xbs")
        mm1_inst = nc.tensor.matmul(xbs[:], M, xsum[:], start=True, stop=True)
        # sel_a (s_last + treat) right after: it heads the positive-branch
        # chain and needs neither pwb nor the psw dot
        sel_a = ps.tile([128, 2], f32, tag="sel_a")
        nc.tensor.matmul(sel_a[:], M, stg[:, 0:2], start=True, stop=True)
        add_dep_helper(pwb_inst.ins, mm1_inst.ins, sync=True,
                       reason="pin PE order: psw broadcast after xbs")

        # ---- per-partition ACT bias tiles straight from PSUM xbs
        nun = len(act_mun) + len(act_lvn)
        biasT = sb.tile([128, nun + 2], f32, tag="biasT")
        bcol = 0
        unit_bias = []
        for a, c, sgn in act_mun + act_lvn:
            nc.vector.tensor_scalar(biasT[:, bcol:bcol + 1], xbs[:],
                                    a / XD, float(c), Alu.mult, Alu.add)
            unit_bias.append(bcol)
            bcol += 1
        J_LV, J_MN = bcol, bcol + 1
        nc.vector.tensor_scalar(biasT[:, J_LV:J_LV + 1], xbs[:],
                                b_lvn / XD, float(a_lvn), Alu.mult, Alu.add)
        nc.vector.tensor_scalar(biasT[:, J_MN:J_MN + 1], xbs[:],
                                b_mun / XD, float(a_mun), Alu.mult, Alu.add)
        xbar = sb.tile([128, 1], f32, tag="xbar")
        xbar_inst = nc.vector.tensor_scalar(xbar[:], xbs[:], 1.0 / XD, None,
                                            Alu.mult)
        # ================= Scalar: relu units, tanh, exp+C ===============
        acc = sb.tile([128, 4], f32, tag="acc")   # cols: A,B,C,D
        relu_ts = []
        for idx, (a, c, sgn) in enumerate(act_mun):
            t = sb.tile([128, HC], f32, tag=f"mn_u{idx}")
            nc.scalar.activation(t[:], tx[:], Act.Relu,
                                 bias=biasT[:, unit_bias[idx]:
                                            unit_bias[idx] + 1],
                                 scale=float(-a / XD))
            relu_ts.append((t, sgn))
        lvn_relu = []
        for idx, (a, c, sgn) in enumerate(act_lvn):
            j = unit_bias[len(act_mun) + idx]
            t = sb.tile([128, HC], f32, tag=f"lv_u{idx}")
            nc.scalar.activation(t[:], tx[:], Act.Relu,
                                 bias=biasT[:, j:j + 1], scale=float(-a / XD))
            lvn_relu.append((t, sgn))

        # tanh input affine emitted explicitly (the compiler would other-
        # wise materialize it late on DVE and stall the ScalarE queue)
        lva = sb.tile([128, HC], f32, tag="lva")
        lva_inst = nc.vector.tensor_scalar(lva[:], tx[:], -b_lvn / XD,
                                           biasT[:, J_LV:J_LV + 1],
                                           Alu.mult, Alu.add)
        cur = lva
        for idx, (t, sgn) in enumerate(lvn_relu):
            nxt = sb.tile([128, HC], f32, tag=f"lv_c{idx}")
            lva_inst = nc.vector.tensor_tensor(nxt[:], cur[:], t[:],
                                               Alu.add if sgn > 0
                                               else Alu.subtract)
            cur = nxt
        lvn = sb.tile([128, HC], f32, tag="lvn")
        tanh_inst = nc.scalar.activation(lvn[:], cur[:], Act.Tanh,
                                         bias=bc(0.0), scale=1.0)
        # the propensity dot rides DVE right after the tanh-input affine:
        # early enough for the sel chain, late enough not to clock-block
        # the tanh
        junkT = sb.tile([128, HC], f32, tag="junkT")
        with nc.allow_low_precision(reason="bf16 pair-sum moving, 0.4% ok"):
            junk_inst = nc.vector.scalar_tensor_tensor(
                junkT[:], tx[:], 1.0, pwb[:], Alu.mult, Alu.mult,
                accum_out=stg[:, 2:3])
        add_dep_helper(junk_inst.ins, lva_inst.ins, sync=True,
                       reason="clock: psw dot after tanh affine")
        sel_b = ps.tile([128, 1], f32, tag="sel_b")
        nc.tensor.matmul(sel_b[:], M, stg[:, 2:3], start=True, stop=True)

        # D-reduce on DVE in parallel with the Exp; C rides the Exp accum
        nc.vector.tensor_reduce(acc[:, 3:4], lvn[:], mybir.AxisListType.X,
                                Alu.add)
        ev = sb.tile([128, HC], f32, tag="ev")
        ev_inst = nc.scalar.activation(ev[:], lvn[:], Act.Exp, bias=bc(-LN2),
                                       scale=-1.0, accum_out=acc[:, 2:3])

        # ---- mun tile on DVE; pinned after the Tanh so the Tanh's
        # scheduled-order clock doesn't include these DVE ops (em waits
        # for ev anyway, so this costs nothing)
        aff = sb.tile([128, HC], f32, tag="aff")
        aff_inst = nc.vector.tensor_scalar(aff[:], tx[:], -b_mun / XD,
                                           biasT[:, J_MN:J_MN + 1],
                                           Alu.mult, Alu.add)
        add_dep_helper(aff_inst.ins, tanh_inst.ins, sync=True,
                       reason="clock: keep mun chain after tanh")
        mun = aff
        for idx, (t, sgn) in enumerate(relu_ts):
            nxt = sb.tile([128, HC], f32, tag=f"mn_c{idx}")
            nc.vector.tensor_tensor(nxt[:], mun[:], t[:],
                                    Alu.add if sgn > 0 else Alu.subtract)
            mun = nxt
        # zero the excluded (i=xd-1) slot so em/emm skip it; fix C directly
        # (the D-col fix folds into dpart below)
        nc.gpsimd.memset(mun[BL:128, HC - 1:HC], 0.0)
        nc.gpsimd.tensor_tensor(acc[BL:128, 2:3], acc[BL:128, 2:3],
                                ev[BL:128, HC - 1:HC], Alu.subtract)

        # ---- A,B accumulations
        em = sb.tile([128, HC], f32, tag="em")
        nc.vector.scalar_tensor_tensor(em[:], ev[:], -2.0, mun[:],
                                       Alu.mult, Alu.mult,
                                       accum_out=acc[:, 1:2])
        emm = sb.tile([128, HC], f32, tag="emm")
        nc.vector.scalar_tensor_tensor(emm[:], em[:], -0.5, mun[:],
                                       Alu.mult, Alu.mult,
                                       accum_out=acc[:, 0:1])

        # ================= positive branch =================
        # xz2 = [xbar | s_last] in bf16; one DVE copy expands it to the
        # (in, unit) = [64, 2*2*7] transpose stationary
        xz2 = sb.tile([BL, 2], bf16, tag="xz2")
        with nc.allow_low_precision(reason="bf16 transpose, 0.4% ok"):
            nc.gpsimd.tensor_copy(xz2[:, 0:1], xbar[0:BL, 0:1])
            nc.vector.tensor_tensor(xz2[:, 1:2], sel_a[0:BL, 0:1],
                                    xbar[0:BL, 0:1], Alu.add)
        # epr = e^{-(z+ps_b)}; sigmoid would force a mid-kernel ACT-table
        # swap (2x 1.3us), so stick to Exp which shares the loaded table
        epr = sb.tile([128, 1], f32, tag="epr")
        nc.scalar.activation(epr[:], sel_b[:, 0:1], Act.Exp,
                             bias=bc(-fc['ps_b']), scale=-1.0)
        xzb = xz2[:].unsqueeze(1).unsqueeze(3).broadcast_to([BL, 2, 2, H])
        xz = sb.tile([BL, 4 * H], bf16, tag="xz")
        with nc.allow_low_precision(reason="bf16 transpose, 0.4% ok"):
            nc.vector.tensor_copy(xz[:], xzb)
        zt = ps.tile([4 * H, BL], bf16, tag="zt")
        nc.tensor.transpose(zt[:], xz[:], tm[0:BL, 0:BL])
        # hpos on DVE (2 ops) keeps ScalarE free for the lvn chain
        h1 = sb.tile([4 * H, BL], f32, tag="h1")
        nc.vector.tensor_scalar(h1[:], zt[:],
                                tc[0:4 * H, lay['posa']:lay['posa'] + 1],
                                tc[0:4 * H, lay['posc']:lay['posc'] + 1],
                                Alu.mult, Alu.add)
        hpos = sb.tile([4 * H, BL], bf16, tag="hpos")
        with nc.allow_low_precision(reason="bf16 mlvp matmul, 0.4% ok"):
            nc.vector.tensor_scalar(hpos[:], h1[:], 0.0, None, Alu.max)
        mlvp = ps.tile([BL, 4], f32, tag="mlvp")
        nc.tensor.matmul(mlvp[:], hpos[:], tm[0:4 * H, 128:132],
                         start=True, stop=True)
        mlv_mu = sb.tile([BL, 2], f32, tag="mlv_mu")
        nc.vector.tensor_scalar(mlv_mu[:], mlvp[:, 0:2], 1.0, fc['b2_mu'],
                                Alu.mult, Alu.add)
        mlv_lv = sb.tile([BL, 2], f32, tag="mlv_lv")
        nc.scalar.activation(mlv_lv[:], mlvp[:, 2:4], Act.Tanh,
                             bias=bc(fc['b2_lv'], 0, BL), scale=1.0)
        ge2 = sb.tile([BL, 2], f32, tag="ge2")
        ge2_inst = nc.scalar.activation(ge2[:], mlv_lv[:], Act.Exp,
                                        bias=bc(LNG, 0, BL), scale=-1.0)

        # ================= F128 chain =================
        F = sb.tile([128, 4], f32, tag="F")
        nc.vector.tensor_scalar(F[:, 0:1], sel_a[:, 1:2], 0.0, None,
                                Alu.is_equal)
        nc.vector.tensor_scalar(F[:, 2:3], sel_a[:, 1:2], 1.0, None,
                                Alu.is_equal)
        den1 = sb.tile([128, 1], f32, tag="den1")
        nc.gpsimd.tensor_scalar(den1[:], epr[:], 1e-4, 1.0 + 1e-4,
                                Alu.mult, Alu.add)
        den0 = sb.tile([128, 1], f32, tag="den0")
        nc.gpsimd.tensor_scalar(den0[:], epr[:], 1.0 + 1e-4, 1e-4,
                                Alu.mult, Alu.add)
        fn0 = sb.tile([128, 1], f32, tag="fn0")
        nc.gpsimd.tensor_scalar(fn0[:], epr[:], 1.0, None, Alu.add)
        r1 = sb.tile([128, 1], f32, tag="r1")
        nc.vector.reciprocal(r1[:], den1[:])
        r0 = sb.tile([128, 1], f32, tag="r0")
        nc.vector.reciprocal(r0[:], den0[:])
        w1v = sb.tile([128, 1], f32, tag="w1v")
        nc.gpsimd.tensor_tensor(w1v[:], fn0[:], r1[:], Alu.mult)
        w0v = sb.tile([128, 1], f32, tag="w0v")
        nc.gpsimd.tensor_tensor(w0v[:], fn0[:], r0[:], Alu.mult)
        nc.gpsimd.tensor_tensor(F[:, 1:2], F[:, 0:1], w0v[:], Alu.mult)
        nc.gpsimd.tensor_tensor(F[:, 3:4], F[:, 2:3], w1v[:], Alu.mult)

        # ======== pos fold: R[0:64,1:] += -G1*(y-mu1)^2 - 255.5*lv1 =====
        # ======== R[0:64,0]  += -G0*(y0-mu0)^2 - 255.5*lv0        =====
        # the -255.5*lv1 term folds into dpart (all cols); only three
        # small adds remain after ge2
        dmu = sb.tile([BL, K], f32, tag="dmu")
        nc.gpsimd.tensor_scalar(dmu[:], ty[0:BL, :], mlv_mu[:, 1:2], None,
                                Alu.subtract)
        dsq = sb.tile([BL, K], f32, tag="dsq")
        nc.gpsimd.tensor_tensor(dsq[:], dmu[:], dmu[:], Alu.mult)
        e0 = sb.tile([BL, 1], f32, tag="e0")
        nc.gpsimd.tensor_scalar(e0[:], ty[0:BL, 0:1], mlv_mu[:, 0:1], None,
                                Alu.subtract)
        e0s = sb.tile([BL, 1], f32, tag="e0s")
        nc.gpsimd.tensor_tensor(e0s[:], e0[:], e0[:], Alu.mult)
        lvq = sb.tile([BL, 1], f32, tag="lvq")
        nc.gpsimd.tensor_scalar(lvq[:], mlv_lv[:, 1:2], -GHALF, None,
                                Alu.mult)
        lv0q = sb.tile([BL, 1], f32, tag="lv0q")
        nc.gpsimd.tensor_scalar(lv0q[:], mlv_lv[:, 0:1], -GHALF, None,
                                Alu.mult)
        ge2n = sb.tile([BL, 1], f32, tag="ge2n")
        nc.vector.tensor_scalar(ge2n[:], ge2[:, 1:2], -1.0, None, Alu.mult)
        g0n = sb.tile([BL, 1], f32, tag="g0n")
        nc.vector.tensor_scalar(g0n[:], ge2[:, 0:1], -1.0, None, Alu.mult)
        # ================= assemble R and finish =================
        yt2 = sb.tile([128, K], f32, tag="yt2")
        nc.gpsimd.tensor_tensor(yt2[:], ty[:], ty[:], Alu.mult)
        R = sb.tile([128, K + 1], f32, tag="R")
        nc.gpsimd.memset(R[0:BL, K:K + 1], 1.0)
        nc.gpsimd.memset(R[BL:128, K:K + 1], 0.0)

        # dpart = 0.5*D with excluded-col fix; all of S1/S2/R-final use
        # the raw accumulated quadratic (no ge2 dependence) so they run
        # as soon as the accums land.
        dpart = sb.tile([128, 1], f32, tag="dpart")
        nc.vector.tensor_scalar(dpart[:], acc[:, 3:4], 0.5, None, Alu.mult)
        nc.vector.scalar_tensor_tensor(dpart[BL:128, 0:1],
                                       lvn[BL:128, HC - 1:HC], -0.5,
                                       dpart[BL:128, 0:1], Alu.mult, Alu.add)
        S1 = sb.tile([128, K], f32, tag="S1")
        nc.vector.tensor_scalar(S1[:], yt2[:], acc[:, 2:3], dpart[:],
                                Alu.mult, Alu.add)
        S2 = sb.tile([128, K], f32, tag="S2")
        nc.vector.scalar_tensor_tensor(S2[:], ty[:], acc[:, 1:2],
                                       S1[:], Alu.mult, Alu.add)
        nc.vector.tensor_scalar(R[:, 0:K], S2[:], 1.0, acc[:, 0:1],
                                Alu.mult, Alu.add)
        # pos fold lands last: cols 1: use the loo path (mu1/lv1/G1),
        # col 0 uses the base path (mu0/lv0/G0)
        nc.vector.scalar_tensor_tensor(
            R[0:BL, 1:K], dsq[:, 1:K], ge2n[:],
            R[0:BL, 1:K], Alu.mult, Alu.add)
        nc.vector.scalar_tensor_tensor(
            R[0:BL, 1:K], lvq[:, 0:1].broadcast_to([BL, K - 1]), 1.0,
            R[0:BL, 1:K], Alu.mult, Alu.add)
        nc.vector.scalar_tensor_tensor(R[0:BL, 0:1], e0s[:], g0n[:],
                                       R[0:BL, 0:1], Alu.mult, Alu.add)
        nc.vector.scalar_tensor_tensor(R[0:BL, 0:1], lv0q[:], 1.0,
                                       R[0:BL, 0:1], Alu.mult, Alu.add)

        P = ps.tile([4, K + 1], f32, tag="P")
        nc.tensor.matmul(P[:], F[:], R[:], start=True, stop=True)
        outs = sb.tile([4, K + 1], f32, tag="outs")
        nc.vector.tensor_copy(outs[:], P[:])
        nc.sync.dma_start(out_d, outs[:])

    nc.compile()
    return nc


def _host_inputs(inputs, fc, spec, lay):
    x = np.ascontiguousarray(inputs['x_samples'], dtype=np.float32)
    y = np.ascontiguousarray(inputs['y_samples'], dtype=np.float32)
    ps_w = inputs['ps_w'].astype(np.float32)[:, 0]

    # psw rows + partition-broadcast stationary [2, 128], bf16 for 1-pass PE
    from ml_dtypes import bfloat16
    pw = np.zeros((2, HC + 128), np.float32)
    pw[0, 0:HC] = ps_w[0:HC]
    pw[1, 0:HC - 1] = ps_w[HC:N1]
    pw[0, HC:HC + BL] = 1.0
    pw[1, HC + BL:HC + 128] = 1.0
    pw = pw.astype(bfloat16)

    Mx = np.zeros((128, 132), np.float32)
    idx = np.arange(128)
    Mx[idx, idx] = 1.0
    Mx[idx ^ 64, idx] = 1.0
    w2sel = np.zeros((4 * H, 4), np.float32)
    w2sel[0:H, 0] = fc['w2_mu']
    w2sel[H:2 * H, 1] = fc['w2_mu']
    w2sel[2 * H:3 * H, 2] = fc['w2_lv']
    w2sel[3 * H:4 * H, 3] = fc['w2_lv']
    Mx[0:4 * H, 128:132] = w2sel
    mb = Mx.astype(bfloat16)

    consts = np.zeros((128, lay['width']), np.float32)
    for i, v in enumerate(lay['bias_vals']):
        consts[:, lay['bias0'] + i] = v
    consts[0:BL, lay['iden']:lay['iden'] + BL] = np.eye(BL, dtype=np.float32)
    posa = np.zeros(4 * H); posc = np.zeros(4 * H)
    posa[0:H] = fc['u_mu'];          posc[0:H] = fc['vb_mu']
    posa[H:2 * H] = fc['u_mu'];      posc[H:2 * H] = fc['vc_mu']
    posa[2 * H:3 * H] = fc['u_lv'];  posc[2 * H:3 * H] = fc['vb_lv']
    posa[3 * H:4 * H] = fc['u_lv'];  posc[3 * H:4 * H] = fc['vc_lv']
    consts[0:4 * H, lay['posa']] = posa
    consts[0:4 * H, lay['posc']] = posc
    w2sel = np.zeros((4 * H, 4), np.float32)
    w2sel[0:H, 0] = fc['w2_mu']
    w2sel[H:2 * H, 1] = fc['w2_mu']
    w2sel[2 * H:3 * H, 2] = fc['w2_lv']
    w2sel[3 * H:4 * H, 3] = fc['w2_lv']
    consts[0:4 * H, lay['w2sel']:lay['w2sel'] + 4] = w2sel

    in_maps = []
    for i in range(NCORES):
        xs = x[i * BL:(i + 1) * BL]                       # [64, 512]
        xt = np.ascontiguousarray(
            xs.reshape(BL, 2, HC).transpose(1, 0, 2).reshape(128, HC))
        ys = y[i * BL:(i + 1) * BL]
        yv = np.ascontiguousarray(np.vstack([ys, ys]))    # [128, K]
        in_maps.append({
            'xt': xt, 'yv': yv, 'pw': pw, 'mb': mb, 'consts': consts,
        })
    return in_maps


def _combine(parts):
    tot = np.zeros((4, K + 1), np.float64)
    for p in parts:
        tot += p.astype(np.float64)
    P0, n0 = tot[0, :K], tot[0, K]
    Q0, r0 = tot[1, :K], tot[1, K]
    P1, n1 = tot[2, :K], tot[2, K]
    Q1, r1 = tot[3, :K], tot[3, K]
    d0 = n0 * (XD - 1)
    d1 = n1 * (XD - 1)
    cmi0 = P0 / d0
    cmi1 = P1 / d1
    dr = 0.5 * ((XD - 1) * cmi0 * (n0 - r0) + Q0) / d0 \
       + 0.5 * ((XD - 1) * cmi1 * (n1 - r1) + Q1) / d1
    cmi_dims = (np.abs(cmi0 + cmi1) / 2.0).astype(np.float32)
    drs = np.abs(dr).astype(np.float32)
    return cmi_dims, drs


def _param_key(inputs, spec):
    import hashlib
    hsh = hashlib.sha256()
    for k in sorted(inputs):
        if k in ('x_samples', 'y_samples'):
            continue
        hsh.update(k.encode())
        hsh.update(np.ascontiguousarray(inputs[k]).tobytes())
    hsh.update(repr(spec).encode())
    return hsh.hexdigest()


def kernel(**inputs):
    from concourse.bass_utils import run_bass_kernel_spmd

    fc = _fold_consts(inputs)
    spec = _specialize(fc, np.asarray(inputs['x_samples']))
    lay = _const_layout(fc, spec)
    key = _param_key(inputs, spec)
    if key not in _prog_cache:
        _prog_cache[key] = _build_program(fc, spec, lay)
    nc = _prog_cache[key]

    in_maps = _host_inputs(inputs, fc, spec, lay)
    res = run_bass_kernel_spmd(nc, in_maps, core_ids=list(range(NCORES)))
    parts = [r['out'] for r in res.results]
    return _combine(parts)



# revision 9
# speedup vs baseline: 1.0496x; 1.0496x over previous
"""Trainium2 Bass kernel for nn_DR_CML (data-parallel over batch, 8 cores).

Math: xm[b,i,j] = x[b,i]*lm_w[j] + lm_b[j] means every row of `loo` is a
linear function of the scalar s[b,i] = xbar[b] - x[b,i]/xd.  The tiny
H=7 MLPs applied to loo collapse to scalar piecewise-linear functions of
s, and sum_i over the [B,K,xd-1] diff tensor collapses to a quadratic in
y with per-row coefficients.  positive[b,k] is itself a quadratic in
y_k, so 511*positive folds into the same per-row quadratic (rows 0:64
only), with a small col-0 delta for the k=0 base-path override:
    R[p,k] = q2[p]*y^2 + q1[p]*y + q0[p]   (+ pos-fold on lower rows)
    P[c,k] = sum_p F128[p,c] * R[p,k]      (one PE matmul, pair-sum free)
with F128 = [f0 | f0*w0 | f1 | f1*w1] computed on all 128 partitions
(the pair-sum matmul with the full M stationary makes per-partition
values equal across halves).

v2 scheduling changes vs the v1 kernel (same math):
  - One fused pair-sum matmul X3 = M @ [xsum | s_raw | treat].
  - No mun-slot memset: the excluded i=xd-1 column is subtracted from
    the A/B/C accumulators post-hoc with narrow GpSimd ops, so `em`
    starts the moment the Exp lands (v1 lost ~1.5us to a GpSimd queue
    hazard here).
  - The mun affine rides ScalarE as an Identity activation (per-
    partition bias AP), freeing a wide DVE slot.
  - h1/hpos and all [*,1] arithmetic (F chain with AluOp.divide, pos
    scalars, fixes) live on GpSimd; DVE keeps only the serial wide
    chain xsum->lva->combine->junk->Dred->em->emm plus [*,K] tiles.
  - R assembled in two partition-disjoint ops; the pos fold is a
    pre-built pf tile absorbed by the row-0:64 STT, not serial adds.
  - y and the consts ride one merged DMA; out DMAs straight from PSUM.

Layout: x is repacked [2*(B/8), xd/2] = [128, 256]; per-row sums are
halved per partition and pair-summed with one PE matmul against M
(M[p,i]=1 iff i==p or i==p^64).  Each core emits a [4,33] tile of
masked partial sums; the host sums 8 tiles and applies the final
formula.
"""
import math

import numpy as np

B, XD, K, H = 512, 512, 32, 7
NCORES = 8
BL = B // NCORES          # 64 rows per core
HC = XD // 2              # 256 columns after repack
N1 = XD - 1
LN2 = math.log(2.0)
LNG = math.log((XD - 1) / 2.0)   # ge2 bias: exp(-lv + LNG) = 255.5*e^-lv

_prog_cache = {}


def _fold_consts(p):
    """Fold linear_map + MLP weights into scalar-MLP coefficients (f64)."""
    lm_w = p['lm_w'].astype(np.float64)
    lm_b = p['lm_b'].astype(np.float64)
    c = lm_b * (XD - 1) / XD

    def fold(w1, b1):
        u = lm_w @ w1.astype(np.float64)
        v_base = lm_b @ w1.astype(np.float64) + b1.astype(np.float64)
        v_c = c @ w1.astype(np.float64) + b1.astype(np.float64)
        return u, v_base, v_c

    u_mu, vb_mu, vc_mu = fold(p['mu_w1'], p['mu_b1'])
    u_lv, vb_lv, vc_lv = fold(p['lv_w1'], p['lv_b1'])
    u_mun, _, vc_mun = fold(p['mun_w1'], p['mun_b1'])
    u_lvn, _, vc_lvn = fold(p['lvn_w1'], p['lvn_b1'])

    return {
        'u_mu': u_mu, 'vb_mu': vb_mu, 'vc_mu': vc_mu,
        'u_lv': u_lv, 'vb_lv': vb_lv, 'vc_lv': vc_lv,
        'u_mun': u_mun, 'vc_mun': vc_mun,
        'u_lvn': u_lvn, 'vc_lvn': vc_lvn,
        'w2_mu': p['mu_w2'][:, 0].astype(np.float64),
        'w2_lv': p['lv_w2'][:, 0].astype(np.float64),
        'w2_mun': p['mun_w2'][:, 0].astype(np.float64),
        'w2_lvn': p['lvn_w2'][:, 0].astype(np.float64),
        'b2_mu': float(p['mu_b2'][0]), 'b2_lv': float(p['lv_b2'][0]),
        'b2_mun': float(p['mun_b2'][0]), 'b2_lvn': float(p['lvn_b2'][0]),
        'ps_b': float(p['ps_b'][0]),
    }


def _specialize(fc, x):
    """Exact per-call relu pruning over the data's s range (i <= xd-2)."""
    x64 = x.astype(np.float64)
    xbar = x64.mean(1)
    s = xbar[:, None] - x64[:, :N1] / XD
    smin, smax = float(s.min()), float(s.max())

    out = {}
    for name, u_all, v_all, w2_all, b2 in (
            ('mun', fc['u_mun'], fc['vc_mun'], fc['w2_mun'], fc['b2_mun']),
            ('lvn', fc['u_lvn'], fc['vc_lvn'], fc['w2_lvn'], fc['b2_lvn'])):
        alpha, beta = b2, 0.0
        active = []
        for u, v, w2 in zip(u_all, v_all, w2_all):
            if w2 == 0.0:
                continue
            lo = min(u * smin, u * smax) + v
            hi = max(u * smin, u * smax) + v
            if lo >= 0.0:          # linear over the data
                alpha += w2 * v
                beta += w2 * u
            elif hi <= 0.0:        # identically zero over the data
                pass
            else:                  # genuinely piecewise on the data
                active.append((float(abs(w2) * u), float(abs(w2) * v),
                               1.0 if w2 > 0 else -1.0))
        out[name] = (float(alpha), float(beta), active)
    return out


def _const_layout(fc, spec):
    """Column layout of the merged [y | consts] f32 tensor.

    Cols 0:K are y; then the bias-value table, then posa/posc.
    """
    bias_vals = [0.0, -LN2, LNG, -fc['ps_b'], fc['b2_lv']]
    seen, ordered = set(), []
    for v in bias_vals:
        if v not in seen:
            seen.add(v)
            ordered.append(v)
    nb = len(ordered)
    lay = {
        'bias_vals': ordered,
        'bias0': K,
        'posa': K + nb,
        'posc': K + nb + 1,
        'width': K + nb + 2,
    }
    return lay


def _build_program(fc, spec, lay):
    from contextlib import ExitStack
    import concourse.tile as tile
    from concourse import bacc, mybir
    from concourse.tile import add_dep_helper

    f32 = mybir.dt.float32
    Alu = mybir.AluOpType
    Act = mybir.ActivationFunctionType

    nc = bacc.Bacc("TRN2", target_bir_lowering=False, debug=False,
                   num_devices=NCORES)

    bf16 = mybir.dt.bfloat16
    xt_d = nc.dram_tensor("xt", [2 * BL, HC], f32, kind="ExternalInput").ap()
    yc_d = nc.dram_tensor("yc", [2 * BL, lay['width']], f32,
                          kind="ExternalInput").ap()
    pw_d = nc.dram_tensor("pw", [2, HC + 128], bf16,
                          kind="ExternalInput").ap()
    mb_d = nc.dram_tensor("mb", [128, 132], bf16, kind="ExternalInput").ap()
    out_d = nc.dram_tensor("out", [4, K + 1], f32, kind="ExternalOutput").ap()

    bias_idx = {v: lay['bias0'] + i for i, v in enumerate(lay['bias_vals'])}
    a_mun, b_mun, act_mun = spec['mun']
    a_lvn, b_lvn, act_lvn = spec['lvn']
    GHALF = float((XD - 1) / 2.0)   # 255.5

    with tile.TileContext(nc) as tcx, ExitStack() as ctx:
        sb = ctx.enter_context(tcx.tile_pool(name="sb", bufs=1))
        ps = ctx.enter_context(tcx.tile_pool(name="ps", bufs=1, space="PSUM"))

        # ---- DMAs: x halves first on both HWDGE queues, small tensors
        # behind them (sync: x-low, M, pw; scalar: x-up, y+consts)
        tx = sb.tile([128, HC], f32, tag="tx")
        nc.sync.dma_start(tx[0:BL, :], xt_d[0:BL, :])
        nc.scalar.dma_start(tx[BL:128, :], xt_d[BL:128, :])
        tm = sb.tile([128, 132], bf16, tag="tm")
        nc.sync.dma_start(tm[:], mb_d)
        tyc = sb.tile([128, lay['width']], f32, tag="tyc")
        nc.scalar.dma_start(tyc[:], yc_d)
        tpw = sb.tile([2, HC + 128], bf16, tag="tpw")
        nc.sync.dma_start(tpw[:], pw_d)

        ty = tyc[:, 0:K]

        # hoist the ACT table load before any data arrives
        warm = sb.tile([1, 1], f32, tag="warm")
        nc.scalar.activation(warm[:], nc.const_aps.tensor(0.0, (1, 1)),
                             Act.Exp, bias=0.0, scale=1.0)
        gwarm = sb.tile([1, 1], f32, tag="gwarm")
        nc.gpsimd.tensor_scalar(gwarm[:], nc.const_aps.tensor(0.0, (1, 1)),
                                1.0, None, Alu.add)

        def bc(val, p0=0, p1=128):
            j = bias_idx[val]
            return tyc[p0:p1, j:j + 1]

        M = tm[:, 0:128]

        # ---- stg assembly: [xsum | s_raw | treat]; cols 1:2 via gpsimd
        stg = sb.tile([128, 3], bf16, tag="stg")
        nc.gpsimd.memset(stg[0:BL, 1:3], 0.0)
        with nc.allow_low_precision(reason="bf16 pair-sum moving, 0.4% ok"):
            nc.gpsimd.tensor_scalar(stg[BL:128, 1:2],
                                    tx[BL:128, HC - 2:HC - 1],
                                    -1.0 / XD, None, Alu.mult)
            nc.gpsimd.tensor_copy(stg[BL:128, 2:3], tx[BL:128, HC - 1:HC])
            nc.vector.tensor_reduce(stg[:, 0:1], tx[:], mybir.AxisListType.X,
                                    Alu.add)

        # ---- one fused pair-sum matmul (full M stationary -> 128 rows)
        X3 = ps.tile([128, 3], f32, tag="X3")
        x3_inst = nc.tensor.matmul(X3[:], M, stg[:], start=True, stop=True)
        # psw partition-broadcast after the pair-sum on PE
        pwb = ps.tile([128, HC], f32, tag="pwb")
        pwb_inst = nc.tensor.matmul(pwb[:], tpw[0:2, HC:HC + 128],
                                    tpw[0:2, 0:HC], start=True, stop=True)
        add_dep_helper(pwb_inst.ins, x3_inst.ins, sync=True,
                       reason="pin PE order: psw broadcast after X3")

        # ---- per-partition ACT bias tiles straight from PSUM X3
        nun = len(act_mun) + len(act_lvn)
        biasT = sb.tile([128, nun + 2], f32, tag="biasT")
        bcol = 0
        unit_bias = []
        for a, c, sgn in act_mun + act_lvn:
            nc.vector.tensor_scalar(biasT[:, bcol:bcol + 1], X3[:, 0:1],
                                    a / XD, float(c), Alu.mult, Alu.add)
            unit_bias.append(bcol)
            bcol += 1
        J_LV, J_MN = bcol, bcol + 1
        nc.vector.tensor_scalar(biasT[:, J_LV:J_LV + 1], X3[:, 0:1],
                                b_lvn / XD, float(a_lvn), Alu.mult, Alu.add)
        nc.vector.tensor_scalar(biasT[:, J_MN:J_MN + 1], X3[:, 0:1],
                                b_mun / XD, float(a_mun), Alu.mult, Alu.add)
        xbar = sb.tile([128, 1], f32, tag="xbar")
        nc.vector.tensor_scalar(xbar[:], X3[:, 0:1], 1.0 / XD, None,
                                Alu.mult)

        # ================= ScalarE: relu units, mun affine ===============
        relu_ts = []
        for idx, (a, c, sgn) in enumerate(act_mun):
            t = sb.tile([128, HC], f32, tag=f"mn_u{idx}")
            nc.scalar.activation(t[:], tx[:], Act.Relu,
                                 bias=biasT[:, unit_bias[idx]:
                                            unit_bias[idx] + 1],
                                 scale=float(-a / XD))
            relu_ts.append((t, sgn))
        lvn_relu = []
        for idx, (a, c, sgn) in enumerate(act_lvn):
            j = unit_bias[len(act_mun) + idx]
            t = sb.tile([128, HC], f32, tag=f"lv_u{idx}")
            nc.scalar.activation(t[:], tx[:], Act.Relu,
                                 bias=biasT[:, j:j + 1], scale=float(-a / XD))
            lvn_relu.append((t, sgn))
        # mun affine on ScalarE (Identity shares the loaded exp table)
        aff = sb.tile([128, HC], f32, tag="aff")
        nc.scalar.activation(aff[:], tx[:], Act.Identity,
                             bias=biasT[:, J_MN:J_MN + 1],
                             scale=float(-b_mun / XD))
        mun = aff
        for idx, (t, sgn) in enumerate(relu_ts):
            nxt = sb.tile([128, HC], f32, tag=f"mn_c{idx}")
            nc.vector.tensor_tensor(nxt[:], mun[:], t[:],
                                    Alu.add if sgn > 0 else Alu.subtract)
            mun = nxt

        # ================= DVE: lva, combine =================
        lva = sb.tile([128, HC], f32, tag="lva")
        nc.vector.tensor_scalar(lva[:], tx[:], -b_lvn / XD,
                                biasT[:, J_LV:J_LV + 1], Alu.mult, Alu.add)
        cur = lva
        for idx, (t, sgn) in enumerate(lvn_relu):
            nxt = sb.tile([128, HC], f32, tag=f"lv_c{idx}")
            nc.vector.tensor_tensor(nxt[:], cur[:], t[:],
                                    Alu.add if sgn > 0
                                    else Alu.subtract)
            cur = nxt
        lvn = sb.tile([128, HC], f32, tag="lvn")
        nc.scalar.activation(lvn[:], cur[:], Act.Tanh, bias=bc(0.0),
                             scale=1.0)

        # propensity dot on DVE right after the combine
        junkT = sb.tile([128, HC], f32, tag="junkT")
        pdd = sb.tile([128, 1], bf16, tag="pdd")
        with nc.allow_low_precision(reason="bf16 pair-sum moving, 0.4% ok"):
            nc.vector.scalar_tensor_tensor(
                junkT[:], tx[:], 1.0, pwb[:], Alu.mult, Alu.mult,
                accum_out=pdd[:])
        sel_b = ps.tile([128, 1], f32, tag="sel_b")
        nc.tensor.matmul(sel_b[:], M, pdd[:], start=True, stop=True)

        # D-reduce on DVE in parallel with the Exp
        acc = sb.tile([128, 4], f32, tag="acc")   # cols: A,B,C,D
        nc.vector.tensor_reduce(acc[:, 3:4], lvn[:], mybir.AxisListType.X,
                                Alu.add)
        ev = sb.tile([128, HC], f32, tag="ev")
        nc.scalar.activation(ev[:], lvn[:], Act.Exp, bias=bc(-LN2),
                             scale=-1.0, accum_out=acc[:, 2:3])
        # epr + positive-branch ACTs queue behind the Exp
        epr = sb.tile([128, 1], f32, tag="epr")
        nc.scalar.activation(epr[:], sel_b[:, 0:1], Act.Exp,
                             bias=bc(-fc['ps_b']), scale=-1.0)

        # ---- A,B accumulations (no mun-slot memset: fixes below)
        em = sb.tile([128, HC], f32, tag="em")
        nc.vector.scalar_tensor_tensor(em[:], ev[:], -2.0, mun[:],
                                       Alu.mult, Alu.mult,
                                       accum_out=acc[:, 1:2])
        emm = sb.tile([128, HC], f32, tag="emm")
        nc.vector.scalar_tensor_tensor(emm[:], em[:], -0.5, mun[:],
                                       Alu.mult, Alu.mult,
                                       accum_out=acc[:, 0:1])
        # excluded i=xd-1 column: narrow GpSimd subtracts off the accums
        nc.gpsimd.tensor_tensor(acc[BL:128, 2:3], acc[BL:128, 2:3],
                                ev[BL:128, HC - 1:HC], Alu.subtract)
        nc.gpsimd.tensor_tensor(acc[BL:128, 1:2], acc[BL:128, 1:2],
                                em[BL:128, HC - 1:HC], Alu.subtract)
        nc.gpsimd.tensor_tensor(acc[BL:128, 0:1], acc[BL:128, 0:1],
                                emm[BL:128, HC - 1:HC], Alu.subtract)
        dpart = sb.tile([128, 1], f32, tag="dpart")
        nc.gpsimd.tensor_scalar(dpart[:], acc[:, 3:4], 0.5, None, Alu.mult)
        dfx = sb.tile([128, 1], f32, tag="dfx")
        nc.gpsimd.tensor_scalar(dfx[BL:128, :], lvn[BL:128, HC - 1:HC],
                                -0.5, None, Alu.mult)
        nc.gpsimd.tensor_tensor(dpart[BL:128, 0:1], dpart[BL:128, 0:1],
                                dfx[BL:128, :], Alu.add)

        # ================= positive branch =================
        xz2 = sb.tile([BL, 2], bf16, tag="xz2")
        with nc.allow_low_precision(reason="bf16 transpose, 0.4% ok"):
            nc.gpsimd.tensor_copy(xz2[:, 0:1], xbar[0:BL, 0:1])
            nc.vector.tensor_tensor(xz2[:, 1:2], X3[0:BL, 1:2],
                                    xbar[0:BL, 0:1], Alu.add)
        xzb = xz2[:].unsqueeze(1).unsqueeze(3).broadcast_to([BL, 2, 2, H])
        xz = sb.tile([BL, 4 * H], bf16, tag="xz")
        with nc.allow_low_precision(reason="bf16 transpose, 0.4% ok"):
            nc.gpsimd.tensor_copy(xz[:], xzb)
        zt = ps.tile([4 * H, BL], bf16, tag="zt")
        nc.tensor.transpose(zt[:], xz[:], tm[0:BL, 0:BL])
        # h1 on DVE (GpSimd cannot read PSUM); hpos on GpSimd
        h1 = sb.tile([4 * H, BL], f32, tag="h1")
        nc.vector.tensor_scalar(h1[:], zt[:],
                                tyc[0:4 * H, lay['posa']:lay['posa'] + 1],
                                tyc[0:4 * H, lay['posc']:lay['posc'] + 1],
                                Alu.mult, Alu.add)
        hpos = sb.tile([4 * H, BL], bf16, tag="hpos")
        with nc.allow_low_precision(reason="bf16 mlvp matmul, 0.4% ok"):
            nc.gpsimd.tensor_scalar(hpos[:], h1[:], 0.0, None, Alu.max)
        mlvp = ps.tile([BL, 4], f32, tag="mlvp")
        nc.tensor.matmul(mlvp[:], hpos[:], tm[0:4 * H, 128:132],
                         start=True, stop=True)
        mlv_mu = sb.tile([BL, 2], f32, tag="mlv_mu")
        nc.vector.tensor_scalar(mlv_mu[:], mlvp[:, 0:2], 1.0, fc['b2_mu'],
                                Alu.mult, Alu.add)
        mlv_lv = sb.tile([BL, 2], f32, tag="mlv_lv")
        nc.scalar.activation(mlv_lv[:], mlvp[:, 2:4], Act.Tanh,
                             bias=bc(fc['b2_lv'], 0, BL), scale=1.0)
        ge2 = sb.tile([BL, 2], f32, tag="ge2")
        nc.scalar.activation(ge2[:], mlv_lv[:], Act.Exp,
                             bias=bc(LNG, 0, BL), scale=-1.0)

        # ================= F128 chain (GpSimd) =================
        F = sb.tile([128, 4], f32, tag="F")
        nc.vector.tensor_scalar(F[:, 0:1], X3[:, 2:3], 0.0, None,
                                Alu.is_equal)
        nc.vector.tensor_scalar(F[:, 2:3], X3[:, 2:3], 1.0, None,
                                Alu.is_equal)
        den1 = sb.tile([128, 1], f32, tag="den1")
        nc.gpsimd.tensor_scalar(den1[:], epr[:], 1e-4, 1.0 + 1e-4,
                                Alu.mult, Alu.add)
        den0 = sb.tile([128, 1], f32, tag="den0")
        nc.gpsimd.tensor_scalar(den0[:], epr[:], 1.0 + 1e-4, 1e-4,
                                Alu.mult, Alu.add)
        fn0 = sb.tile([128, 1], f32, tag="fn0")
        nc.gpsimd.tensor_scalar(fn0[:], epr[:], 1.0, None, Alu.add)
        r1 = sb.tile([128, 1], f32, tag="r1")
        nc.vector.reciprocal(r1[:], den1[:])
        r0 = sb.tile([128, 1], f32, tag="r0")
        nc.vector.reciprocal(r0[:], den0[:])
        w1v = sb.tile([128, 1], f32, tag="w1v")
        nc.gpsimd.tensor_tensor(w1v[:], fn0[:], r1[:], Alu.mult)
        w0v = sb.tile([128, 1], f32, tag="w0v")
        nc.gpsimd.tensor_tensor(w0v[:], fn0[:], r0[:], Alu.mult)
        nc.gpsimd.tensor_tensor(F[:, 1:2], F[:, 0:1], w0v[:], Alu.mult)
        nc.gpsimd.tensor_tensor(F[:, 3:4], F[:, 2:3], w1v[:], Alu.mult)

        # ---- pos-fold scalars (GpSimd narrow)
        e0 = sb.tile([BL, 1], f32, tag="e0")
        nc.gpsimd.tensor_scalar(e0[:], ty[0:BL, 0:1], mlv_mu[:, 0:1], None,
                                Alu.subtract)
        e0s = sb.tile([BL, 1], f32, tag="e0s")
        nc.gpsimd.tensor_tensor(e0s[:], e0[:], e0[:], Alu.mult)
        lvq = sb.tile([BL, 1], f32, tag="lvq")
        nc.gpsimd.tensor_scalar(lvq[:], mlv_lv[:, 1:2], -GHALF, None,
                                Alu.mult)
        lv0q = sb.tile([BL, 1], f32, tag="lv0q")
        nc.gpsimd.tensor_scalar(lv0q[:], mlv_lv[:, 0:1], -GHALF, None,
                                Alu.mult)
        ge2n = sb.tile([BL, 1], f32, tag="ge2n")
        nc.gpsimd.tensor_scalar(ge2n[:], ge2[:, 1:2], -1.0, None, Alu.mult)
        g0n = sb.tile([BL, 1], f32, tag="g0n")
        nc.gpsimd.tensor_scalar(g0n[:], ge2[:, 0:1], -1.0, None, Alu.mult)
        # pf col0: base-path override at k=0
        pf = sb.tile([BL, K], f32, tag="pf")
        pg0 = sb.tile([BL, 1], f32, tag="pg0")
        nc.gpsimd.tensor_tensor(pg0[:], e0s[:], g0n[:], Alu.mult)
        nc.gpsimd.tensor_tensor(pf[:, 0:1], pg0[:], lv0q[:], Alu.add)

        # ---- [*,K] tiles: yt2 early on GpSimd, dmu/dsq on DVE
        yt2 = sb.tile([128, K], f32, tag="yt2")
        nc.gpsimd.tensor_tensor(yt2[:], ty[:], ty[:], Alu.mult)
        dmu = sb.tile([BL, K], f32, tag="dmu")
        nc.vector.tensor_scalar(dmu[:], ty[0:BL, :], mlv_mu[:, 1:2], None,
                                Alu.subtract)
        dsq = sb.tile([BL, K], f32, tag="dsq")
        nc.vector.tensor_tensor(dsq[:], dmu[:], dmu[:], Alu.mult)
        # pf cols 1:: loo-path quadratic in y
        nc.vector.scalar_tensor_tensor(
            pf[:, 1:K], dsq[:, 1:K], ge2n[:],
            lvq[:, 0:1].broadcast_to([BL, K - 1]), Alu.mult, Alu.add)

        # ================= R assembly and finish =================
        R = sb.tile([128, K + 1], f32, tag="R")
        nc.gpsimd.memset(R[0:BL, K:K + 1], 1.0)
        nc.gpsimd.memset(R[BL:128, K:K + 1], 0.0)
        S1 = sb.tile([128, K], f32, tag="S1")
        nc.vector.tensor_scalar(S1[:], yt2[:], acc[:, 2:3], dpart[:],
                                Alu.mult, Alu.add)
        S2 = sb.tile([128, K], f32, tag="S2")
        nc.vector.scalar_tensor_tensor(S2[:], ty[:], acc[:, 1:2],
                                       S1[:], Alu.mult, Alu.add)
        nc.vector.tensor_scalar(R[BL:128, 0:K], S2[BL:128, :], 1.0,
                                acc[BL:128, 0:1], Alu.mult, Alu.add)
        nc.vector.scalar_tensor_tensor(R[0:BL, 0:K], pf[:],
                                       acc[0:BL, 0:1], S2[0:BL, :],
                                       Alu.add, Alu.add)

        P = ps.tile([4, K + 1], f32, tag="P")
        nc.tensor.matmul(P[:], F[:], R[:], start=True, stop=True)
        outs = sb.tile([4, K + 1], f32, tag="outs")
        nc.vector.tensor_copy(outs[:], P[:])
        nc.sync.dma_start(out_d, outs[:])

    nc.compile()
    return nc


def _host_inputs(inputs, fc, spec, lay):
    x = np.ascontiguousarray(inputs['x_samples'], dtype=np.float32)
    y = np.ascontiguousarray(inputs['y_samples'], dtype=np.float32)
    ps_w = inputs['ps_w'].astype(np.float32)[:, 0]

    # psw rows + partition-broadcast stationary [2, 128], bf16 for 1-pass PE
    from ml_dtypes import bfloat16
    pw = np.zeros((2, HC + 128), np.float32)
    pw[0, 0:HC] = ps_w[0:HC]
    pw[1, 0:HC - 1] = ps_w[HC:N1]
    pw[0, HC:HC + BL] = 1.0
    pw[1, HC + BL:HC + 128] = 1.0
    pw = pw.astype(bfloat16)

    Mx = np.zeros((128, 132), np.float32)
    idx = np.arange(128)
    Mx[idx, idx] = 1.0
    Mx[idx ^ 64, idx] = 1.0
    w2sel = np.zeros((4 * H, 4), np.float32)
    w2sel[0:H, 0] = fc['w2_mu']
    w2sel[H:2 * H, 1] = fc['w2_mu']
    w2sel[2 * H:3 * H, 2] = fc['w2_lv']
    w2sel[3 * H:4 * H, 3] = fc['w2_lv']
    Mx[0:4 * H, 128:132] = w2sel
    mb = Mx.astype(bfloat16)

    consts = np.zeros((128, lay['width'] - K), np.float32)
    for i, v in enumerate(lay['bias_vals']):
        consts[:, lay['bias0'] - K + i] = v
    posa = np.zeros(4 * H); posc = np.zeros(4 * H)
    posa[0:H] = fc['u_mu'];          posc[0:H] = fc['vb_mu']
    posa[H:2 * H] = fc['u_mu'];      posc[H:2 * H] = fc['vc_mu']
    posa[2 * H:3 * H] = fc['u_lv'];  posc[2 * H:3 * H] = fc['vb_lv']
    posa[3 * H:4 * H] = fc['u_lv'];  posc[3 * H:4 * H] = fc['vc_lv']
    consts[0:4 * H, lay['posa'] - K] = posa
    consts[0:4 * H, lay['posc'] - K] = posc

    in_maps = []
    for i in range(NCORES):
        xs = x[i * BL:(i + 1) * BL]                       # [64, 512]
        xt = np.ascontiguousarray(
            xs.reshape(BL, 2, HC).transpose(1, 0, 2).reshape(128, HC))
        ys = y[i * BL:(i + 1) * BL]
        yv = np.ascontiguousarray(np.vstack([ys, ys]))    # [128, K]
        yc = np.ascontiguousarray(
            np.hstack([yv, consts]).astype(np.float32))   # [128, width]
        in_maps.append({
            'xt': xt, 'yc': yc, 'pw': pw, 'mb': mb,
        })
    return in_maps


def _combine(parts):
    tot = np.zeros((4, K + 1), np.float64)
    for p in parts:
        tot += p.astype(np.float64)
    P0, n0 = tot[0, :K], tot[0, K]
    Q0, r0 = tot[1, :K], tot[1, K]
    P1, n1 = tot[2, :K], tot[2, K]
    Q1, r1 = tot[3, :K], tot[3, K]
    d0 = n0 * (XD - 1)
    d1 = n1 * (XD - 1)
    cmi0 = P0 / d0
    cmi1 = P1 / d1
    dr = 0.5 * ((XD - 1) * cmi0 * (n0 - r0) + Q0) / d0 \
       + 0.5 * ((XD - 1) * cmi1 * (n1 - r1) + Q1) / d1
    cmi_dims = (np.abs(cmi0 + cmi1) / 2.0).astype(np.float32)
    drs = np.abs(dr).astype(np.float32)
    return cmi_dims, drs


def _param_key(inputs, spec):
    import hashlib
    hsh = hashlib.sha256()
    for k in sorted(inputs):
        if k in ('x_samples', 'y_samples'):
            continue
        hsh.update(k.encode())
        hsh.update(np.ascontiguousarray(inputs[k]).tobytes())
    hsh.update(repr(spec).encode())
    return hsh.hexdigest()


def kernel(**inputs):
    from concourse.bass_utils import run_bass_kernel_spmd

    fc = _fold_consts(inputs)
    spec = _specialize(fc, np.asarray(inputs['x_samples']))
    lay = _const_layout(fc, spec)
    key = _param_key(inputs, spec)
    if key not in _prog_cache:
        _prog_cache[key] = _build_program(fc, spec, lay)
    nc = _prog_cache[key]

    in_maps = _host_inputs(inputs, fc, spec, lay)
    res = run_bass_kernel_spmd(nc, in_maps, core_ids=list(range(NCORES)))
    parts = [r['out'] for r in res.results]
    return _combine(parts)


# revision 14
# speedup vs baseline: 1.2321x; 1.1738x over previous
"""Trainium2 Bass kernel for nn_DR_CML (data-parallel over batch, 8 cores).

Math: xm[b,i,j] = x[b,i]*lm_w[j] + lm_b[j] means every row of `loo` is a
linear function of the scalar s[b,i] = xbar[b] - x[b,i]/xd.  The tiny
H=7 MLPs applied to loo collapse to scalar piecewise-linear functions of
s, and sum_i over the [B,K,xd-1] diff tensor collapses to a quadratic in
y with per-row coefficients.  positive[b,k] is itself a quadratic in
y_k, so 511*positive folds into the same per-row quadratic (rows 0:64
only), with a small col-0 delta for the k=0 base-path override:
    R[p,k] = q2[p]*y^2 + q1[p]*y + q0[p]   (+ pos-fold on lower rows)
    P[c,k] = sum_p F128[p,c] * R[p,k]      (one PE matmul, pair-sum free)
with F128 = [f0 | f0*w0 | f1 | f1*w1] computed on all 128 partitions
(the pair-sum matmul with the full M stationary makes per-partition
values equal across halves).

v2 scheduling changes vs the v1 kernel (same math):
  - One fused pair-sum matmul X3 = M @ [xsum | s_raw | treat].
  - No mun-slot memset: the excluded i=xd-1 column is subtracted from
    the A/B/C accumulators post-hoc with narrow GpSimd ops, so `em`
    starts the moment the Exp lands (v1 lost ~1.5us to a GpSimd queue
    hazard here).
  - The mun affine rides ScalarE as an Identity activation (per-
    partition bias AP), freeing a wide DVE slot.
  - h1/hpos and all [*,1] arithmetic (F chain with AluOp.divide, pos
    scalars, fixes) live on GpSimd; DVE keeps only the serial wide
    chain xsum->lva->combine->junk->Dred->em->emm plus [*,K] tiles.
  - R assembled in two partition-disjoint ops; the pos fold is a
    pre-built pf tile absorbed by the row-0:64 STT, not serial adds.
  - y and the consts ride one merged DMA; out DMAs straight from PSUM.

Layout: x is repacked [2*(B/8), xd/2] = [128, 256]; per-row sums are
halved per partition and pair-summed with one PE matmul against M
(M[p,i]=1 iff i==p or i==p^64).  Each core emits a [4,33] tile of
masked partial sums; the host sums 8 tiles and applies the final
formula.
"""
import math

import numpy as np

B, XD, K, H = 512, 512, 32, 7
NCORES = 8
BL = B // NCORES          # 64 rows per core
HC = XD // 2              # 256 columns after repack
N1 = XD - 1
LN2 = math.log(2.0)
LNG = math.log((XD - 1) / 2.0)   # ge2 bias: exp(-lv + LNG) = 255.5*e^-lv

_prog_cache = {}


def _fold_consts(p):
    """Fold linear_map + MLP weights into scalar-MLP coefficients (f64)."""
    lm_w = p['lm_w'].astype(np.float64)
    lm_b = p['lm_b'].astype(np.float64)
    c = lm_b * (XD - 1) / XD

    def fold(w1, b1):
        u = lm_w @ w1.astype(np.float64)
        v_base = lm_b @ w1.astype(np.float64) + b1.astype(np.float64)
        v_c = c @ w1.astype(np.float64) + b1.astype(np.float64)
        return u, v_base, v_c

    u_mu, vb_mu, vc_mu = fold(p['mu_w1'], p['mu_b1'])
    u_lv, vb_lv, vc_lv = fold(p['lv_w1'], p['lv_b1'])
    u_mun, _, vc_mun = fold(p['mun_w1'], p['mun_b1'])
    u_lvn, _, vc_lvn = fold(p['lvn_w1'], p['lvn_b1'])

    return {
        'u_mu': u_mu, 'vb_mu': vb_mu, 'vc_mu': vc_mu,
        'u_lv': u_lv, 'vb_lv': vb_lv, 'vc_lv': vc_lv,
        'u_mun': u_mun, 'vc_mun': vc_mun,
        'u_lvn': u_lvn, 'vc_lvn': vc_lvn,
        'w2_mu': p['mu_w2'][:, 0].astype(np.float64),
        'w2_lv': p['lv_w2'][:, 0].astype(np.float64),
        'w2_mun': p['mun_w2'][:, 0].astype(np.float64),
        'w2_lvn': p['lvn_w2'][:, 0].astype(np.float64),
        'b2_mu': float(p['mu_b2'][0]), 'b2_lv': float(p['lv_b2'][0]),
        'b2_mun': float(p['mun_b2'][0]), 'b2_lvn': float(p['lvn_b2'][0]),
        'ps_b': float(p['ps_b'][0]),
    }


def _specialize(fc, x):
    """Exact per-call relu pruning over the data's s range (i <= xd-2)."""
    x64 = x.astype(np.float64)
    xbar = x64.mean(1)
    s = xbar[:, None] - x64[:, :N1] / XD
    smin, smax = float(s.min()), float(s.max())

    out = {}
    for name, u_all, v_all, w2_all, b2 in (
            ('mun', fc['u_mun'], fc['vc_mun'], fc['w2_mun'], fc['b2_mun']),
            ('lvn', fc['u_lvn'], fc['vc_lvn'], fc['w2_lvn'], fc['b2_lvn'])):
        alpha, beta = b2, 0.0
        active = []
        for u, v, w2 in zip(u_all, v_all, w2_all):
            if w2 == 0.0:
                continue
            lo = min(u * smin, u * smax) + v
            hi = max(u * smin, u * smax) + v
            if lo >= 0.0:          # linear over the data
                alpha += w2 * v
                beta += w2 * u
            elif hi <= 0.0:        # identically zero over the data
                pass
            else:                  # genuinely piecewise on the data
                active.append((float(abs(w2) * u), float(abs(w2) * v),
                               1.0 if w2 > 0 else -1.0))
        out[name] = (float(alpha), float(beta), active)
    return out


def _const_layout(fc, spec):
    """Column layout of the merged [y | consts] f32 tensor.

    Cols 0:K are y; then the bias-value table, then posa/posc.
    """
    bias_vals = [0.0, -LN2, LNG, -fc['ps_b'], fc['b2_lv']]
    seen, ordered = set(), []
    for v in bias_vals:
        if v not in seen:
            seen.add(v)
            ordered.append(v)
    nb = len(ordered)
    lay = {
        'bias_vals': ordered,
        'bias0': K,
        'posa': K + nb,
        'posc': K + nb + 1,
        'eq01': K + nb + 2,    # 2 cols: [0.0, 1.0] for the F masks
        'cA': K + nb + 4,      # 3 cols: E-scale  [1e-4, 1+1e-4, 1.0]
        'cB': K + nb + 7,      # 3 cols: E-offset [1+1e-4, 1e-4, 1.0]
        'width': K + nb + 10,
    }
    return lay


def _build_program(fc, spec, lay):
    from contextlib import ExitStack
    import concourse.tile as tile
    from concourse import bacc, mybir
    from concourse.tile import add_dep_helper

    f32 = mybir.dt.float32
    Alu = mybir.AluOpType
    Act = mybir.ActivationFunctionType

    nc = bacc.Bacc("TRN2", target_bir_lowering=False, debug=False,
                   num_devices=NCORES)

    bf16 = mybir.dt.bfloat16
    xt_d = nc.dram_tensor("xt", [2 * BL, HC], f32, kind="ExternalInput").ap()
    yc_d = nc.dram_tensor("yc", [2 * BL, lay['width']], f32,
                          kind="ExternalInput").ap()
    pw_d = nc.dram_tensor("pw", [2, HC + 128], bf16,
                          kind="ExternalInput").ap()
    mb_d = nc.dram_tensor("mb", [128, 132], bf16, kind="ExternalInput").ap()
    out_d = nc.dram_tensor("out", [4, K + 1], f32, kind="ExternalOutput").ap()

    bias_idx = {v: lay['bias0'] + i for i, v in enumerate(lay['bias_vals'])}
    a_mun, b_mun, act_mun = spec['mun']
    a_lvn, b_lvn, act_lvn = spec['lvn']
    GHALF = float((XD - 1) / 2.0)   # 255.5

    with tile.TileContext(nc) as tcx, ExitStack() as ctx:
        sb = ctx.enter_context(tcx.tile_pool(name="sb", bufs=1))
        ps = ctx.enter_context(tcx.tile_pool(name="ps", bufs=1, space="PSUM"))

        # ---- DMAs: x halves first on both HWDGE queues, small tensors
        # behind them (sync: x-low, M, pw; scalar: x-up, y+consts)
        tx = sb.tile([128, HC], f32, tag="tx")
        nc.sync.dma_start(tx[0:BL, :], xt_d[0:BL, :])
        nc.scalar.dma_start(tx[BL:128, :], xt_d[BL:128, :])
        tm = sb.tile([128, 132], bf16, tag="tm")
        nc.sync.dma_start(tm[:], mb_d)
        tyc = sb.tile([128, lay['width']], f32, tag="tyc")
        nc.scalar.dma_start(tyc[:], yc_d)
        tpw = sb.tile([2, HC + 128], bf16, tag="tpw")
        nc.sync.dma_start(tpw[:], pw_d)

        ty = tyc[:, 0:K]

        # hoist the ACT table load before any data arrives
        warm = sb.tile([1, 1], f32, tag="warm")
        nc.scalar.activation(warm[:], nc.const_aps.tensor(0.0, (1, 1)),
                             Act.Exp, bias=0.0, scale=1.0)
        gwarm = sb.tile([1, 1], f32, tag="gwarm")
        nc.gpsimd.tensor_scalar(gwarm[:], nc.const_aps.tensor(0.0, (1, 1)),
                                1.0, None, Alu.add)

        def bc(val, p0=0, p1=128):
            j = bias_idx[val]
            return tyc[p0:p1, j:j + 1]

        M = tm[:, 0:128]

        # ---- stg assembly: [xsum | s_raw | treat]; cols 1:2 via gpsimd
        stg = sb.tile([128, 3], bf16, tag="stg")
        nc.gpsimd.memset(stg[0:BL, 1:3], 0.0)
        with nc.allow_low_precision(reason="bf16 pair-sum moving, 0.4% ok"):
            nc.gpsimd.tensor_scalar(stg[BL:128, 1:2],
                                    tx[BL:128, HC - 2:HC - 1],
                                    -1.0 / XD, None, Alu.mult)
            nc.gpsimd.tensor_copy(stg[BL:128, 2:3], tx[BL:128, HC - 1:HC])
            nc.vector.tensor_reduce(stg[:, 0:1], tx[:], mybir.AxisListType.X,
                                    Alu.add)

        # ---- one fused pair-sum matmul (full M stationary -> 128 rows)
        X3 = ps.tile([128, 3], f32, tag="X3")
        x3_inst = nc.tensor.matmul(X3[:], M, stg[:], start=True, stop=True)
        # psw partition-broadcast after the pair-sum on PE
        pwb = ps.tile([128, HC], f32, tag="pwb")
        pwb_inst = nc.tensor.matmul(pwb[:], tpw[0:2, HC:HC + 128],
                                    tpw[0:2, 0:HC], start=True, stop=True)
        add_dep_helper(pwb_inst.ins, x3_inst.ins, sync=True,
                       reason="pin PE order: psw broadcast after X3")

        # ---- per-partition ACT bias tiles straight from PSUM X3
        nun = len(act_mun) + len(act_lvn)
        biasT = sb.tile([128, nun + 2], f32, tag="biasT")
        bcol = 0
        unit_bias = []
        for a, c, sgn in act_mun + act_lvn:
            nc.vector.tensor_scalar(biasT[:, bcol:bcol + 1], X3[:, 0:1],
                                    a / XD, float(c), Alu.mult, Alu.add)
            unit_bias.append(bcol)
            bcol += 1
        J_LV, J_MN = bcol, bcol + 1
        nc.vector.tensor_scalar(biasT[:, J_LV:J_LV + 1], X3[:, 0:1],
                                b_lvn / XD, float(a_lvn), Alu.mult, Alu.add)
        nc.vector.tensor_scalar(biasT[:, J_MN:J_MN + 1], X3[:, 0:1],
                                b_mun / XD, float(a_mun), Alu.mult, Alu.add)
        xbar = sb.tile([128, 1], f32, tag="xbar")
        nc.vector.tensor_scalar(xbar[:], X3[:, 0:1], 1.0 / XD, None,
                                Alu.mult)
        # F masks early: [f0 | f1] in one TT against the [0,1] const row
        F = sb.tile([128, 4], f32, tag="F")
        nc.vector.tensor_tensor(F[:, 0:2],
                                X3[:, 2:3].broadcast_to([128, 2]),
                                tyc[:, lay['eq01']:lay['eq01'] + 2],
                                Alu.is_equal)

        # ================= ScalarE: relu units, mun affine ===============
        relu_ts = []
        for idx, (a, c, sgn) in enumerate(act_mun):
            t = sb.tile([128, HC], f32, tag=f"mn_u{idx}")
            nc.scalar.activation(t[:], tx[:], Act.Relu,
                                 bias=biasT[:, unit_bias[idx]:
                                            unit_bias[idx] + 1],
                                 scale=float(-a / XD))
            relu_ts.append((t, sgn))
        lvn_relu = []
        for idx, (a, c, sgn) in enumerate(act_lvn):
            j = unit_bias[len(act_mun) + idx]
            t = sb.tile([128, HC], f32, tag=f"lv_u{idx}")
            nc.scalar.activation(t[:], tx[:], Act.Relu,
                                 bias=biasT[:, j:j + 1], scale=float(-a / XD))
            lvn_relu.append((t, sgn))
        # mun affine on ScalarE (Identity shares the loaded exp table)
        aff = sb.tile([128, HC], f32, tag="aff")
        nc.scalar.activation(aff[:], tx[:], Act.Identity,
                             bias=biasT[:, J_MN:J_MN + 1],
                             scale=float(-b_mun / XD))
        mun = aff
        for idx, (t, sgn) in enumerate(relu_ts):
            nxt = sb.tile([128, HC], f32, tag=f"mn_c{idx}")
            nc.vector.tensor_tensor(nxt[:], mun[:], t[:],
                                    Alu.add if sgn > 0 else Alu.subtract)
            mun = nxt

        # ================= DVE: lva, combine =================
        lva = sb.tile([128, HC], f32, tag="lva")
        nc.vector.tensor_scalar(lva[:], tx[:], -b_lvn / XD,
                                biasT[:, J_LV:J_LV + 1], Alu.mult, Alu.add)
        cur = lva
        for idx, (t, sgn) in enumerate(lvn_relu):
            nxt = sb.tile([128, HC], f32, tag=f"lv_c{idx}")
            nc.vector.tensor_tensor(nxt[:], cur[:], t[:],
                                    Alu.add if sgn > 0
                                    else Alu.subtract)
            cur = nxt
        lvn = sb.tile([128, HC], f32, tag="lvn")
        nc.scalar.activation(lvn[:], cur[:], Act.Tanh, bias=bc(0.0),
                             scale=1.0)

        # propensity dot on DVE right after the combine
        junkT = sb.tile([128, HC], f32, tag="junkT")
        pdd = sb.tile([128, 1], bf16, tag="pdd")
        with nc.allow_low_precision(reason="bf16 pair-sum moving, 0.4% ok"):
            nc.vector.scalar_tensor_tensor(
                junkT[:], tx[:], 1.0, pwb[:], Alu.mult, Alu.mult,
                accum_out=pdd[:])
        sel_b = ps.tile([128, 1], f32, tag="sel_b")
        nc.tensor.matmul(sel_b[:], M, pdd[:], start=True, stop=True)

        # ================= positive branch (front half) =================
        xz2 = sb.tile([BL, 2], bf16, tag="xz2")
        with nc.allow_low_precision(reason="bf16 transpose, 0.4% ok"):
            nc.gpsimd.tensor_copy(xz2[:, 0:1], xbar[0:BL, 0:1])
            nc.vector.tensor_tensor(xz2[:, 1:2], X3[0:BL, 1:2],
                                    xbar[0:BL, 0:1], Alu.add)
        xzb = xz2[:].unsqueeze(1).unsqueeze(3).broadcast_to([BL, 2, 2, H])
        xz = sb.tile([BL, 4 * H], bf16, tag="xz")
        with nc.allow_low_precision(reason="bf16 transpose, 0.4% ok"):
            nc.gpsimd.tensor_copy(xz[:], xzb)
        zt = ps.tile([4 * H, BL], bf16, tag="zt")
        nc.tensor.transpose(zt[:], xz[:], tm[0:BL, 0:BL])
        # h1/hpos on DVE (GpSimd cannot read PSUM and is slow on [p,64])
        h1 = sb.tile([4 * H, BL], f32, tag="h1")
        nc.vector.tensor_scalar(h1[:], zt[:],
                                tyc[0:4 * H, lay['posa']:lay['posa'] + 1],
                                tyc[0:4 * H, lay['posc']:lay['posc'] + 1],
                                Alu.mult, Alu.add)
        hpos = sb.tile([4 * H, BL], bf16, tag="hpos")
        with nc.allow_low_precision(reason="bf16 mlvp matmul, 0.4% ok"):
            nc.vector.tensor_scalar(hpos[:], h1[:], 0.0, None, Alu.max)
        mlvp = ps.tile([BL, 4], f32, tag="mlvp")
        nc.tensor.matmul(mlvp[:], hpos[:], tm[0:4 * H, 128:132],
                         start=True, stop=True)
        # negated mu pair: the ACT-Square biases (y - mu)^2 need -mu
        mlv_mun = sb.tile([BL, 2], f32, tag="mlv_mun")
        nc.vector.tensor_scalar(mlv_mun[:], mlvp[:, 0:2], -1.0,
                                -fc['b2_mu'], Alu.mult, Alu.add)

        # ---- D-reduce on DVE in the Exp shadow; split accumulators
        accD = sb.tile([128, 1], f32, tag="accD")
        nc.vector.tensor_reduce(accD[:], lvn[:], mybir.AxisListType.X,
                                Alu.add)
        accC = sb.tile([128, 1], f32, tag="accC")
        ev = sb.tile([128, HC], f32, tag="ev")
        ev_inst = nc.scalar.activation(ev[:], lvn[:], Act.Exp, bias=bc(-LN2),
                                       scale=-1.0, accum_out=accC[:])
        # epr after the Exp on ScalarE (pinned so it can't preempt it)
        epr = sb.tile([128, 1], f32, tag="epr")
        epr_inst = nc.scalar.activation(epr[:], sel_b[:, 0:1], Act.Exp,
                                        bias=bc(-fc['ps_b']), scale=-1.0)
        add_dep_helper(epr_inst.ins, ev_inst.ins, sync=True,
                       reason="clock: epr behind the critical Exp")

        # ---- A,B accumulations (no mun-slot memset: fixes below)
        accB = sb.tile([128, 1], f32, tag="accB")
        em = sb.tile([128, HC], f32, tag="em")
        nc.vector.scalar_tensor_tensor(em[:], ev[:], -2.0, mun[:],
                                       Alu.mult, Alu.mult,
                                       accum_out=accB[:])
        accA = sb.tile([128, 1], f32, tag="accA")
        emm = sb.tile([128, HC], f32, tag="emm")
        nc.vector.scalar_tensor_tensor(emm[:], em[:], -0.5, mun[:],
                                       Alu.mult, Alu.mult,
                                       accum_out=accA[:])
        # excluded i=xd-1 column: narrow GpSimd subtracts off the accums
        nc.gpsimd.tensor_tensor(accC[BL:128, :], accC[BL:128, :],
                                ev[BL:128, HC - 1:HC], Alu.subtract)
        nc.gpsimd.tensor_tensor(accB[BL:128, :], accB[BL:128, :],
                                em[BL:128, HC - 1:HC], Alu.subtract)
        nc.gpsimd.tensor_tensor(accA[BL:128, :], accA[BL:128, :],
                                emm[BL:128, HC - 1:HC], Alu.subtract)
        dpart = sb.tile([128, 1], f32, tag="dpart")
        nc.gpsimd.tensor_scalar(dpart[:], accD[:], 0.5, None, Alu.mult)
        dfx = sb.tile([128, 1], f32, tag="dfx")
        nc.gpsimd.tensor_scalar(dfx[BL:128, :], lvn[BL:128, HC - 1:HC],
                                -0.5, None, Alu.mult)
        nc.gpsimd.tensor_tensor(dpart[BL:128, 0:1], dpart[BL:128, 0:1],
                                dfx[BL:128, :], Alu.add)

        # ================= positive branch (back half) =================
        mlv_lv = sb.tile([BL, 2], f32, tag="mlv_lv")
        nc.scalar.activation(mlv_lv[:], mlvp[:, 2:4], Act.Tanh,
                             bias=bc(fc['b2_lv'], 0, BL), scale=1.0)
        ge2 = sb.tile([BL, 2], f32, tag="ge2")
        nc.scalar.activation(ge2[:], mlv_lv[:], Act.Exp,
                             bias=bc(LNG, 0, BL), scale=-1.0)
        # (y - mu)^2 on ScalarE via Square with the -mu bias AP
        dsq = sb.tile([BL, K], f32, tag="dsq")
        nc.scalar.activation(dsq[:], ty[0:BL, :], Act.Square,
                             bias=mlv_mun[:, 1:2], scale=1.0)
        e0s = sb.tile([BL, 1], f32, tag="e0s")
        nc.scalar.activation(e0s[:], ty[0:BL, 0:1], Act.Square,
                             bias=mlv_mun[:, 0:1], scale=1.0)
        # negated [g0n | ge2n] and [lv0q | lvq] pairs on GpSimd
        ge2n2 = sb.tile([BL, 2], f32, tag="ge2n2")
        nc.gpsimd.tensor_scalar(ge2n2[:], ge2[:], -1.0, None, Alu.mult)
        lvq2 = sb.tile([BL, 2], f32, tag="lvq2")
        nc.gpsimd.tensor_scalar(lvq2[:], mlv_lv[:], -GHALF, None, Alu.mult)

        # ================= F128 chain (DVE, batched) =================
        # fdd = E*[1e-4, 1+1e-4, 1] + [1+1e-4, 1e-4, 1]: den1|den0|fn0
        fdd = sb.tile([128, 3], f32, tag="fdd")
        nc.vector.tensor_tensor(fdd[:], epr[:].broadcast_to([128, 3]),
                                tyc[:, lay['cA']:lay['cA'] + 3], Alu.mult)
        nc.vector.tensor_tensor(fdd[:], fdd[:],
                                tyc[:, lay['cB']:lay['cB'] + 3], Alu.add)
        rr = sb.tile([128, 2], f32, tag="rr")
        nc.vector.reciprocal(rr[:], fdd[:, 0:2])
        # F cols 2,3 = [f0*w0 | f1*w1], w = fn0 * 1/den
        nc.vector.scalar_tensor_tensor(F[:, 2:3], F[:, 0:1], fdd[:, 2:3],
                                       rr[:, 0:1], Alu.mult, Alu.mult)
        nc.vector.scalar_tensor_tensor(F[:, 3:4], F[:, 1:2], fdd[:, 2:3],
                                       rr[:, 1:2], Alu.mult, Alu.mult)

        # ---- yt2 early on GpSimd
        yt2 = sb.tile([128, K], f32, tag="yt2")
        nc.gpsimd.tensor_tensor(yt2[:], ty[:], ty[:], Alu.mult)

        # ================= R assembly and finish =================
        R = sb.tile([128, K + 1], f32, tag="R")
        nc.gpsimd.memset(R[0:BL, K:K + 1], 1.0)
        nc.gpsimd.memset(R[BL:128, K:K + 1], 0.0)
        S1 = sb.tile([128, K], f32, tag="S1")
        nc.vector.tensor_scalar(S1[:], yt2[:], accC[:], dpart[:],
                                Alu.mult, Alu.add)
        S2 = sb.tile([128, K], f32, tag="S2")
        nc.vector.scalar_tensor_tensor(S2[:], ty[:], accB[:],
                                       S1[:], Alu.mult, Alu.add)
        nc.vector.tensor_scalar(R[BL:128, 0:K], S2[BL:128, :], 1.0,
                                accA[BL:128, :], Alu.mult, Alu.add)
        # pos fold tile pf: col0 base-path, cols 1: loo-path
        pf = sb.tile([BL, K], f32, tag="pf")
        nc.vector.scalar_tensor_tensor(
            pf[:, 1:K], dsq[:, 1:K], ge2n2[:, 1:2],
            lvq2[:, 1:2].broadcast_to([BL, K - 1]), Alu.mult, Alu.add)
        nc.vector.scalar_tensor_tensor(pf[:, 0:1], e0s[:], ge2n2[:, 0:1],
                                       lvq2[:, 0:1], Alu.mult, Alu.add)
        nc.vector.scalar_tensor_tensor(R[0:BL, 0:K], pf[:],
                                       accA[0:BL, :], S2[0:BL, :],
                                       Alu.add, Alu.add)

        P = ps.tile([4, K + 1], f32, tag="P")
        nc.tensor.matmul(P[:], F[:], R[:], start=True, stop=True)
        outs = sb.tile([4, K + 1], f32, tag="outs")
        nc.vector.tensor_copy(outs[:], P[:])
        nc.sync.dma_start(out_d, outs[:])

    nc.compile()
    return nc


def _host_inputs(inputs, fc, spec, lay):
    x = np.ascontiguousarray(inputs['x_samples'], dtype=np.float32)
    y = np.ascontiguousarray(inputs['y_samples'], dtype=np.float32)
    ps_w = inputs['ps_w'].astype(np.float32)[:, 0]

    # psw rows + partition-broadcast stationary [2, 128], bf16 for 1-pass PE
    from ml_dtypes import bfloat16
    pw = np.zeros((2, HC + 128), np.float32)
    pw[0, 0:HC] = ps_w[0:HC]
    pw[1, 0:HC - 1] = ps_w[HC:N1]
    pw[0, HC:HC + BL] = 1.0
    pw[1, HC + BL:HC + 128] = 1.0
    pw = pw.astype(bfloat16)

    Mx = np.zeros((128, 132), np.float32)
    idx = np.arange(128)
    Mx[idx, idx] = 1.0
    Mx[idx ^ 64, idx] = 1.0
    w2sel = np.zeros((4 * H, 4), np.float32)
    w2sel[0:H, 0] = fc['w2_mu']
    w2sel[H:2 * H, 1] = fc['w2_mu']
    w2sel[2 * H:3 * H, 2] = fc['w2_lv']
    w2sel[3 * H:4 * H, 3] = fc['w2_lv']
    Mx[0:4 * H, 128:132] = w2sel
    mb = Mx.astype(bfloat16)

    consts = np.zeros((128, lay['width'] - K), np.float32)
    for i, v in enumerate(lay['bias_vals']):
        consts[:, lay['bias0'] - K + i] = v
    consts[:, lay['eq01'] - K:lay['eq01'] - K + 2] = [0.0, 1.0]
    consts[:, lay['cA'] - K:lay['cA'] - K + 3] = [1e-4, 1.0 + 1e-4, 1.0]
    consts[:, lay['cB'] - K:lay['cB'] - K + 3] = [1.0 + 1e-4, 1e-4, 1.0]
    posa = np.zeros(4 * H); posc = np.zeros(4 * H)
    posa[0:H] = fc['u_mu'];          posc[0:H] = fc['vb_mu']
    posa[H:2 * H] = fc['u_mu'];      posc[H:2 * H] = fc['vc_mu']
    posa[2 * H:3 * H] = fc['u_lv'];  posc[2 * H:3 * H] = fc['vb_lv']
    posa[3 * H:4 * H] = fc['u_lv'];  posc[3 * H:4 * H] = fc['vc_lv']
    consts[0:4 * H, lay['posa'] - K] = posa
    consts[0:4 * H, lay['posc'] - K] = posc

    in_maps = []
    for i in range(NCORES):
        xs = x[i * BL:(i + 1) * BL]                       # [64, 512]
        xt = np.ascontiguousarray(
            xs.reshape(BL, 2, HC).transpose(1, 0, 2).reshape(128, HC))
        ys = y[i * BL:(i + 1) * BL]
        yv = np.ascontiguousarray(np.vstack([ys, ys]))    # [128, K]
        yc = np.ascontiguousarray(
            np.hstack([yv, consts]).astype(np.float32))   # [128, width]
        in_maps.append({
            'xt': xt, 'yc': yc, 'pw': pw, 'mb': mb,
        })
    return in_maps


def _combine(parts):
    tot = np.zeros((4, K + 1), np.float64)
    for p in parts:
        tot += p.astype(np.float64)
    P0, n0 = tot[0, :K], tot[0, K]
    P1, n1 = tot[1, :K], tot[1, K]
    Q0, r0 = tot[2, :K], tot[2, K]
    Q1, r1 = tot[3, :K], tot[3, K]
    d0 = n0 * (XD - 1)
    d1 = n1 * (XD - 1)
    cmi0 = P0 / d0
    cmi1 = P1 / d1
    dr = 0.5 * ((XD - 1) * cmi0 * (n0 - r0) + Q0) / d0 \
       + 0.5 * ((XD - 1) * cmi1 * (n1 - r1) + Q1) / d1
    cmi_dims = (np.abs(cmi0 + cmi1) / 2.0).astype(np.float32)
    drs = np.abs(dr).astype(np.float32)
    return cmi_dims, drs


def _param_key(inputs, spec):
    import hashlib
    hsh = hashlib.sha256()
    for k in sorted(inputs):
        if k in ('x_samples', 'y_samples'):
            continue
        hsh.update(k.encode())
        hsh.update(np.ascontiguousarray(inputs[k]).tobytes())
    hsh.update(repr(spec).encode())
    return hsh.hexdigest()


def kernel(**inputs):
    from concourse.bass_utils import run_bass_kernel_spmd

    fc = _fold_consts(inputs)
    spec = _specialize(fc, np.asarray(inputs['x_samples']))
    lay = _const_layout(fc, spec)
    key = _param_key(inputs, spec)
    if key not in _prog_cache:
        _prog_cache[key] = _build_program(fc, spec, lay)
    nc = _prog_cache[key]

    in_maps = _host_inputs(inputs, fc, spec, lay)
    res = run_bass_kernel_spmd(nc, in_maps, core_ids=list(range(NCORES)))
    parts = [r['out'] for r in res.results]
    return _combine(parts)


# revision 15
# speedup vs baseline: 1.2391x; 1.0057x over previous
"""Trainium2 Bass kernel for nn_DR_CML (data-parallel over batch, 8 cores).

Math: xm[b,i,j] = x[b,i]*lm_w[j] + lm_b[j] means every row of `loo` is a
linear function of the scalar s[b,i] = xbar[b] - x[b,i]/xd.  The tiny
H=7 MLPs applied to loo collapse to scalar piecewise-linear functions of
s, and sum_i over the [B,K,xd-1] diff tensor collapses to a quadratic in
y with per-row coefficients.  positive[b,k] is itself a quadratic in
y_k, so 511*positive folds into the same per-row quadratic (rows 0:64
only), with a small col-0 delta for the k=0 base-path override:
    R[p,k] = q2[p]*y^2 + q1[p]*y + q0[p]   (+ pos-fold on lower rows)
    P[c,k] = sum_p F128[p,c] * R[p,k]      (one PE matmul, pair-sum free)
with F128 = [f0 | f0*w0 | f1 | f1*w1] computed on all 128 partitions
(the pair-sum matmul with the full M stationary makes per-partition
values equal across halves).

v2 scheduling changes vs the v1 kernel (same math):
  - One fused pair-sum matmul X3 = M @ [xsum | s_raw | treat].
  - No mun-slot memset: the excluded i=xd-1 column is subtracted from
    the A/B/C accumulators post-hoc with narrow GpSimd ops, so `em`
    starts the moment the Exp lands (v1 lost ~1.5us to a GpSimd queue
    hazard here).
  - The mun affine rides ScalarE as an Identity activation (per-
    partition bias AP), freeing a wide DVE slot.
  - h1/hpos and all [*,1] arithmetic (F chain with AluOp.divide, pos
    scalars, fixes) live on GpSimd; DVE keeps only the serial wide
    chain xsum->lva->combine->junk->Dred->em->emm plus [*,K] tiles.
  - R assembled in two partition-disjoint ops; the pos fold is a
    pre-built pf tile absorbed by the row-0:64 STT, not serial adds.
  - y and the consts ride one merged DMA; out DMAs straight from PSUM.

Layout: x is repacked [2*(B/8), xd/2] = [128, 256]; per-row sums are
halved per partition and pair-summed with one PE matmul against M
(M[p,i]=1 iff i==p or i==p^64).  Each core emits a [4,33] tile of
masked partial sums; the host sums 8 tiles and applies the final
formula.
"""
import math

import numpy as np

B, XD, K, H = 512, 512, 32, 7
NCORES = 8
BL = B // NCORES          # 64 rows per core
HC = XD // 2              # 256 columns after repack
N1 = XD - 1
LN2 = math.log(2.0)
LNG = math.log((XD - 1) / 2.0)   # ge2 bias: exp(-lv + LNG) = 255.5*e^-lv

_prog_cache = {}


def _fold_consts(p):
    """Fold linear_map + MLP weights into scalar-MLP coefficients (f64)."""
    lm_w = p['lm_w'].astype(np.float64)
    lm_b = p['lm_b'].astype(np.float64)
    c = lm_b * (XD - 1) / XD

    def fold(w1, b1):
        u = lm_w @ w1.astype(np.float64)
        v_base = lm_b @ w1.astype(np.float64) + b1.astype(np.float64)
        v_c = c @ w1.astype(np.float64) + b1.astype(np.float64)
        return u, v_base, v_c

    u_mu, vb_mu, vc_mu = fold(p['mu_w1'], p['mu_b1'])
    u_lv, vb_lv, vc_lv = fold(p['lv_w1'], p['lv_b1'])
    u_mun, _, vc_mun = fold(p['mun_w1'], p['mun_b1'])
    u_lvn, _, vc_lvn = fold(p['lvn_w1'], p['lvn_b1'])

    return {
        'u_mu': u_mu, 'vb_mu': vb_mu, 'vc_mu': vc_mu,
        'u_lv': u_lv, 'vb_lv': vb_lv, 'vc_lv': vc_lv,
        'u_mun': u_mun, 'vc_mun': vc_mun,
        'u_lvn': u_lvn, 'vc_lvn': vc_lvn,
        'w2_mu': p['mu_w2'][:, 0].astype(np.float64),
        'w2_lv': p['lv_w2'][:, 0].astype(np.float64),
        'w2_mun': p['mun_w2'][:, 0].astype(np.float64),
        'w2_lvn': p['lvn_w2'][:, 0].astype(np.float64),
        'b2_mu': float(p['mu_b2'][0]), 'b2_lv': float(p['lv_b2'][0]),
        'b2_mun': float(p['mun_b2'][0]), 'b2_lvn': float(p['lvn_b2'][0]),
        'ps_b': float(p['ps_b'][0]),
    }


def _specialize(fc, x):
    """Exact per-call relu pruning over the data's s range (i <= xd-2)."""
    x64 = x.astype(np.float64)
    xbar = x64.mean(1)
    s = xbar[:, None] - x64[:, :N1] / XD
    smin, smax = float(s.min()), float(s.max())

    out = {}
    for name, u_all, v_all, w2_all, b2 in (
            ('mun', fc['u_mun'], fc['vc_mun'], fc['w2_mun'], fc['b2_mun']),
            ('lvn', fc['u_lvn'], fc['vc_lvn'], fc['w2_lvn'], fc['b2_lvn'])):
        alpha, beta = b2, 0.0
        active = []
        for u, v, w2 in zip(u_all, v_all, w2_all):
            if w2 == 0.0:
                continue
            lo = min(u * smin, u * smax) + v
            hi = max(u * smin, u * smax) + v
            if lo >= 0.0:          # linear over the data
                alpha += w2 * v
                beta += w2 * u
            elif hi <= 0.0:        # identically zero over the data
                pass
            else:                  # genuinely piecewise on the data
                active.append((float(abs(w2) * u), float(abs(w2) * v),
                               1.0 if w2 > 0 else -1.0))
        out[name] = (float(alpha), float(beta), active)
    return out


def _const_layout(fc, spec):
    """Column layout of the merged [y | consts] f32 tensor.

    Cols 0:K are y; then the bias-value table, then posa/posc.
    """
    bias_vals = [0.0, -LN2, LNG, -fc['ps_b'], fc['b2_lv']]
    seen, ordered = set(), []
    for v in bias_vals:
        if v not in seen:
            seen.add(v)
            ordered.append(v)
    nb = len(ordered)
    lay = {
        'bias_vals': ordered,
        'bias0': K,
        'posa': K + nb,
        'posc': K + nb + 1,
        'eq01': K + nb + 2,    # 2 cols: [0.0, 1.0] for the F masks
        'cA': K + nb + 4,      # 3 cols: E-scale  [1e-4, 1+1e-4, 1.0]
        'cB': K + nb + 7,      # 3 cols: E-offset [1+1e-4, 1e-4, 1.0]
        'width': K + nb + 10,
    }
    return lay


def _build_program(fc, spec, lay):
    from contextlib import ExitStack
    import concourse.tile as tile
    from concourse import bacc, mybir
    from concourse.tile import add_dep_helper

    f32 = mybir.dt.float32
    Alu = mybir.AluOpType
    Act = mybir.ActivationFunctionType

    nc = bacc.Bacc("TRN2", target_bir_lowering=False, debug=False,
                   num_devices=NCORES)

    bf16 = mybir.dt.bfloat16
    xt_d = nc.dram_tensor("xt", [2 * BL, HC], f32, kind="ExternalInput").ap()
    yc_d = nc.dram_tensor("yc", [2 * BL, lay['width']], f32,
                          kind="ExternalInput").ap()
    pw_d = nc.dram_tensor("pw", [2, HC + 128], bf16,
                          kind="ExternalInput").ap()
    mb_d = nc.dram_tensor("mb", [128, 132], bf16, kind="ExternalInput").ap()
    out_d = nc.dram_tensor("out", [4, K + 1], f32, kind="ExternalOutput").ap()

    bias_idx = {v: lay['bias0'] + i for i, v in enumerate(lay['bias_vals'])}
    a_mun, b_mun, act_mun = spec['mun']
    a_lvn, b_lvn, act_lvn = spec['lvn']
    GHALF = float((XD - 1) / 2.0)   # 255.5

    with tile.TileContext(nc) as tcx, ExitStack() as ctx:
        sb = ctx.enter_context(tcx.tile_pool(name="sb", bufs=1))
        ps = ctx.enter_context(tcx.tile_pool(name="ps", bufs=1, space="PSUM"))

        # ---- DMAs: x halves first on both HWDGE queues, small tensors
        # behind them (sync: x-low, M, pw; scalar: x-up, y+consts)
        tx = sb.tile([128, HC], f32, tag="tx")
        nc.sync.dma_start(tx[0:BL, :], xt_d[0:BL, :])
        nc.scalar.dma_start(tx[BL:128, :], xt_d[BL:128, :])
        tm = sb.tile([128, 132], bf16, tag="tm")
        nc.sync.dma_start(tm[:], mb_d)
        tyc = sb.tile([128, lay['width']], f32, tag="tyc")
        nc.scalar.dma_start(tyc[:], yc_d)
        tpw = sb.tile([2, HC + 128], bf16, tag="tpw")
        nc.sync.dma_start(tpw[:], pw_d)

        ty = tyc[:, 0:K]

        # hoist the ACT table load before any data arrives
        warm = sb.tile([1, 1], f32, tag="warm")
        nc.scalar.activation(warm[:], nc.const_aps.tensor(0.0, (1, 1)),
                             Act.Exp, bias=0.0, scale=1.0)
        gwarm = sb.tile([1, 1], f32, tag="gwarm")
        nc.gpsimd.tensor_scalar(gwarm[:], nc.const_aps.tensor(0.0, (1, 1)),
                                1.0, None, Alu.add)

        def bc(val, p0=0, p1=128):
            j = bias_idx[val]
            return tyc[p0:p1, j:j + 1]

        M = tm[:, 0:128]

        # ---- stg assembly: [xsum | s_raw | treat]; cols 1:2 via gpsimd
        stg = sb.tile([128, 3], bf16, tag="stg")
        nc.gpsimd.memset(stg[0:BL, 1:3], 0.0)
        with nc.allow_low_precision(reason="bf16 pair-sum moving, 0.4% ok"):
            nc.gpsimd.tensor_scalar(stg[BL:128, 1:2],
                                    tx[BL:128, HC - 2:HC - 1],
                                    -1.0 / XD, None, Alu.mult)
            nc.gpsimd.tensor_copy(stg[BL:128, 2:3], tx[BL:128, HC - 1:HC])
            nc.vector.tensor_reduce(stg[:, 0:1], tx[:], mybir.AxisListType.X,
                                    Alu.add)

        # ---- one fused pair-sum matmul (full M stationary -> 128 rows)
        X3 = ps.tile([128, 3], f32, tag="X3")
        x3_inst = nc.tensor.matmul(X3[:], M, stg[:], start=True, stop=True)
        # psw partition-broadcast after the pair-sum on PE
        pwb = ps.tile([128, HC], f32, tag="pwb")
        pwb_inst = nc.tensor.matmul(pwb[:], tpw[0:2, HC:HC + 128],
                                    tpw[0:2, 0:HC], start=True, stop=True)
        add_dep_helper(pwb_inst.ins, x3_inst.ins, sync=True,
                       reason="pin PE order: psw broadcast after X3")

        # ---- per-partition ACT bias tiles straight from PSUM X3
        nun = len(act_mun) + len(act_lvn)
        biasT = sb.tile([128, nun + 2], f32, tag="biasT")
        bcol = 0
        unit_bias = []
        for a, c, sgn in act_mun + act_lvn:
            nc.vector.tensor_scalar(biasT[:, bcol:bcol + 1], X3[:, 0:1],
                                    a / XD, float(c), Alu.mult, Alu.add)
            unit_bias.append(bcol)
            bcol += 1
        J_LV, J_MN = bcol, bcol + 1
        nc.vector.tensor_scalar(biasT[:, J_LV:J_LV + 1], X3[:, 0:1],
                                b_lvn / XD, float(a_lvn), Alu.mult, Alu.add)
        nc.vector.tensor_scalar(biasT[:, J_MN:J_MN + 1], X3[:, 0:1],
                                b_mun / XD, float(a_mun), Alu.mult, Alu.add)
        xbar = sb.tile([128, 1], f32, tag="xbar")
        nc.vector.tensor_scalar(xbar[:], X3[:, 0:1], 1.0 / XD, None,
                                Alu.mult)
        # F masks early: [f0 | f1] in one TT against the [0,1] const row
        F = sb.tile([128, 4], f32, tag="F")
        nc.vector.tensor_tensor(F[:, 0:2],
                                X3[:, 2:3].broadcast_to([128, 2]),
                                tyc[:, lay['eq01']:lay['eq01'] + 2],
                                Alu.is_equal)

        # ================= ScalarE: relu units, mun affine ===============
        relu_ts = []
        for idx, (a, c, sgn) in enumerate(act_mun):
            t = sb.tile([128, HC], f32, tag=f"mn_u{idx}")
            nc.scalar.activation(t[:], tx[:], Act.Relu,
                                 bias=biasT[:, unit_bias[idx]:
                                            unit_bias[idx] + 1],
                                 scale=float(-a / XD))
            relu_ts.append((t, sgn))
        lvn_relu = []
        for idx, (a, c, sgn) in enumerate(act_lvn):
            j = unit_bias[len(act_mun) + idx]
            t = sb.tile([128, HC], f32, tag=f"lv_u{idx}")
            nc.scalar.activation(t[:], tx[:], Act.Relu,
                                 bias=biasT[:, j:j + 1], scale=float(-a / XD))
            lvn_relu.append((t, sgn))
        # mun affine on ScalarE (Identity shares the loaded exp table)
        aff = sb.tile([128, HC], f32, tag="aff")
        nc.scalar.activation(aff[:], tx[:], Act.Identity,
                             bias=biasT[:, J_MN:J_MN + 1],
                             scale=float(-b_mun / XD))
        mun = aff
        for idx, (t, sgn) in enumerate(relu_ts):
            nxt = sb.tile([128, HC], f32, tag=f"mn_c{idx}")
            nc.vector.tensor_tensor(nxt[:], mun[:], t[:],
                                    Alu.add if sgn > 0 else Alu.subtract)
            mun = nxt

        # ================= DVE: lva, combine =================
        lva = sb.tile([128, HC], f32, tag="lva")
        nc.vector.tensor_scalar(lva[:], tx[:], -b_lvn / XD,
                                biasT[:, J_LV:J_LV + 1], Alu.mult, Alu.add)
        cur = lva
        for idx, (t, sgn) in enumerate(lvn_relu):
            nxt = sb.tile([128, HC], f32, tag=f"lv_c{idx}")
            nc.vector.tensor_tensor(nxt[:], cur[:], t[:],
                                    Alu.add if sgn > 0
                                    else Alu.subtract)
            cur = nxt
        lvn = sb.tile([128, HC], f32, tag="lvn")
        nc.scalar.activation(lvn[:], cur[:], Act.Tanh, bias=bc(0.0),
                             scale=1.0)

        # propensity dot on DVE right after the combine
        junkT = sb.tile([128, HC], f32, tag="junkT")
        pdd = sb.tile([128, 1], bf16, tag="pdd")
        with nc.allow_low_precision(reason="bf16 pair-sum moving, 0.4% ok"):
            nc.vector.scalar_tensor_tensor(
                junkT[:], tx[:], 1.0, pwb[:], Alu.mult, Alu.mult,
                accum_out=pdd[:])
        sel_b = ps.tile([128, 1], f32, tag="sel_b")
        nc.tensor.matmul(sel_b[:], M, pdd[:], start=True, stop=True)

        # ================= positive branch (front half) =================
        xz2 = sb.tile([BL, 2], bf16, tag="xz2")
        with nc.allow_low_precision(reason="bf16 transpose, 0.4% ok"):
            nc.gpsimd.tensor_copy(xz2[:, 0:1], xbar[0:BL, 0:1])
            nc.vector.tensor_tensor(xz2[:, 1:2], X3[0:BL, 1:2],
                                    xbar[0:BL, 0:1], Alu.add)
        xzb = xz2[:].unsqueeze(1).unsqueeze(3).broadcast_to([BL, 2, 2, H])
        xz = sb.tile([BL, 4 * H], bf16, tag="xz")
        with nc.allow_low_precision(reason="bf16 transpose, 0.4% ok"):
            nc.gpsimd.tensor_copy(xz[:], xzb)
        zt = ps.tile([4 * H, BL], bf16, tag="zt")
        nc.tensor.transpose(zt[:], xz[:], tm[0:BL, 0:BL])
        # h1/hpos on DVE (GpSimd cannot read PSUM and is slow on [p,64])
        h1 = sb.tile([4 * H, BL], f32, tag="h1")
        nc.vector.tensor_scalar(h1[:], zt[:],
                                tyc[0:4 * H, lay['posa']:lay['posa'] + 1],
                                tyc[0:4 * H, lay['posc']:lay['posc'] + 1],
                                Alu.mult, Alu.add)
        hpos = sb.tile([4 * H, BL], bf16, tag="hpos")
        with nc.allow_low_precision(reason="bf16 mlvp matmul, 0.4% ok"):
            nc.vector.tensor_scalar(hpos[:], h1[:], 0.0, None, Alu.max)
        mlvp = ps.tile([BL, 4], f32, tag="mlvp")
        nc.tensor.matmul(mlvp[:], hpos[:], tm[0:4 * H, 128:132],
                         start=True, stop=True)
        # negated mu pair: the ACT-Square biases (y - mu)^2 need -mu
        mlv_mun = sb.tile([BL, 2], f32, tag="mlv_mun")
        nc.vector.tensor_scalar(mlv_mun[:], mlvp[:, 0:2], -1.0,
                                -fc['b2_mu'], Alu.mult, Alu.add)

        # ---- D-reduce on DVE in the Exp shadow; split accumulators
        accD = sb.tile([128, 1], f32, tag="accD")
        nc.vector.tensor_reduce(accD[:], lvn[:], mybir.AxisListType.X,
                                Alu.add)
        accC = sb.tile([128, 1], f32, tag="accC")
        ev = sb.tile([128, HC], f32, tag="ev")
        ev_inst = nc.scalar.activation(ev[:], lvn[:], Act.Exp, bias=bc(-LN2),
                                       scale=-1.0, accum_out=accC[:])
        # epr after the Exp on ScalarE (pinned so it can't preempt it)
        epr = sb.tile([128, 1], f32, tag="epr")
        epr_inst = nc.scalar.activation(epr[:], sel_b[:, 0:1], Act.Exp,
                                        bias=bc(-fc['ps_b']), scale=-1.0)
        add_dep_helper(epr_inst.ins, ev_inst.ins, sync=True,
                       reason="clock: epr behind the critical Exp")

        # ---- A,B accumulations (no mun-slot memset: fixes below)
        accB = sb.tile([128, 1], f32, tag="accB")
        em = sb.tile([128, HC], f32, tag="em")
        nc.vector.scalar_tensor_tensor(em[:], ev[:], -2.0, mun[:],
                                       Alu.mult, Alu.mult,
                                       accum_out=accB[:])
        accA = sb.tile([128, 1], f32, tag="accA")
        emm = sb.tile([128, HC], f32, tag="emm")
        nc.vector.scalar_tensor_tensor(emm[:], em[:], -0.5, mun[:],
                                       Alu.mult, Alu.mult,
                                       accum_out=accA[:])
        # excluded i=xd-1 column: narrow GpSimd subtracts off the accums
        nc.gpsimd.tensor_tensor(accC[BL:128, :], accC[BL:128, :],
                                ev[BL:128, HC - 1:HC], Alu.subtract)
        nc.gpsimd.tensor_tensor(accB[BL:128, :], accB[BL:128, :],
                                em[BL:128, HC - 1:HC], Alu.subtract)
        nc.gpsimd.tensor_tensor(accA[BL:128, :], accA[BL:128, :],
                                emm[BL:128, HC - 1:HC], Alu.subtract)
        dpart = sb.tile([128, 1], f32, tag="dpart")
        nc.gpsimd.tensor_scalar(dpart[:], accD[:], 0.5, None, Alu.mult)
        dfx = sb.tile([128, 1], f32, tag="dfx")
        nc.gpsimd.tensor_scalar(dfx[BL:128, :], lvn[BL:128, HC - 1:HC],
                                -0.5, None, Alu.mult)
        nc.gpsimd.tensor_tensor(dpart[BL:128, 0:1], dpart[BL:128, 0:1],
                                dfx[BL:128, :], Alu.add)

        # ================= positive branch (back half) =================
        mlv_lv = sb.tile([BL, 2], f32, tag="mlv_lv")
        nc.scalar.activation(mlv_lv[:], mlvp[:, 2:4], Act.Tanh,
                             bias=bc(fc['b2_lv'], 0, BL), scale=1.0)
        ge2 = sb.tile([BL, 2], f32, tag="ge2")
        nc.scalar.activation(ge2[:], mlv_lv[:], Act.Exp,
                             bias=bc(LNG, 0, BL), scale=-1.0)
        # (y - mu)^2 on ScalarE via Square with the -mu bias AP
        dsq = sb.tile([BL, K], f32, tag="dsq")
        nc.scalar.activation(dsq[:], ty[0:BL, :], Act.Square,
                             bias=mlv_mun[:, 1:2], scale=1.0)
        e0s = sb.tile([BL, 1], f32, tag="e0s")
        nc.scalar.activation(e0s[:], ty[0:BL, 0:1], Act.Square,
                             bias=mlv_mun[:, 0:1], scale=1.0)
        # negated [g0n | ge2n] and [lv0q | lvq] pairs on GpSimd
        ge2n2 = sb.tile([BL, 2], f32, tag="ge2n2")
        nc.gpsimd.tensor_scalar(ge2n2[:], ge2[:], -1.0, None, Alu.mult)
        lvq2 = sb.tile([BL, 2], f32, tag="lvq2")
        nc.gpsimd.tensor_scalar(lvq2[:], mlv_lv[:], -GHALF, None, Alu.mult)

        # ================= F128 chain (DVE, batched) =================
        # fdd = E*[1e-4, 1+1e-4, 1] + [1+1e-4, 1e-4, 1]: den1|den0|fn0
        fdd = sb.tile([128, 3], f32, tag="fdd")
        nc.vector.tensor_tensor(fdd[:], epr[:].broadcast_to([128, 3]),
                                tyc[:, lay['cA']:lay['cA'] + 3], Alu.mult)
        nc.vector.tensor_tensor(fdd[:], fdd[:],
                                tyc[:, lay['cB']:lay['cB'] + 3], Alu.add)
        rr = sb.tile([128, 2], f32, tag="rr")
        nc.vector.reciprocal(rr[:], fdd[:, 0:2])
        # F cols 2,3 = [f0*w0 | f1*w1]; w0 = fn0/den0, w1 = fn0/den1
        # (fdd col0 = den1, col1 = den0 -> rr col1 is 1/den0)
        nc.vector.scalar_tensor_tensor(F[:, 2:3], F[:, 0:1], fdd[:, 2:3],
                                       rr[:, 1:2], Alu.mult, Alu.mult)
        nc.vector.scalar_tensor_tensor(F[:, 3:4], F[:, 1:2], fdd[:, 2:3],
                                       rr[:, 0:1], Alu.mult, Alu.mult)

        # ---- yt2 early on GpSimd
        yt2 = sb.tile([128, K], f32, tag="yt2")
        nc.gpsimd.tensor_tensor(yt2[:], ty[:], ty[:], Alu.mult)

        # ================= R assembly and finish =================
        R = sb.tile([128, K + 1], f32, tag="R")
        nc.gpsimd.memset(R[0:BL, K:K + 1], 1.0)
        nc.gpsimd.memset(R[BL:128, K:K + 1], 0.0)
        S1 = sb.tile([128, K], f32, tag="S1")
        nc.vector.tensor_scalar(S1[:], yt2[:], accC[:], dpart[:],
                                Alu.mult, Alu.add)
        S2 = sb.tile([128, K], f32, tag="S2")
        nc.vector.scalar_tensor_tensor(S2[:], ty[:], accB[:],
                                       S1[:], Alu.mult, Alu.add)
        nc.vector.tensor_scalar(R[BL:128, 0:K], S2[BL:128, :], 1.0,
                                accA[BL:128, :], Alu.mult, Alu.add)
        # pos fold tile pf: col0 base-path, cols 1: loo-path
        pf = sb.tile([BL, K], f32, tag="pf")
        nc.vector.scalar_tensor_tensor(
            pf[:, 1:K], dsq[:, 1:K], ge2n2[:, 1:2],
            lvq2[:, 1:2].broadcast_to([BL, K - 1]), Alu.mult, Alu.add)
        nc.vector.scalar_tensor_tensor(pf[:, 0:1], e0s[:], ge2n2[:, 0:1],
                                       lvq2[:, 0:1], Alu.mult, Alu.add)
        nc.vector.scalar_tensor_tensor(R[0:BL, 0:K], pf[:],
                                       accA[0:BL, :], S2[0:BL, :],
                                       Alu.add, Alu.add)

        P = ps.tile([4, K + 1], f32, tag="P")
        nc.tensor.matmul(P[:], F[:], R[:], start=True, stop=True)
        outs = sb.tile([4, K + 1], f32, tag="outs")
        nc.vector.tensor_copy(outs[:], P[:])
        nc.sync.dma_start(out_d, outs[:])

    nc.compile()
    return nc


def _host_inputs(inputs, fc, spec, lay):
    x = np.ascontiguousarray(inputs['x_samples'], dtype=np.float32)
    y = np.ascontiguousarray(inputs['y_samples'], dtype=np.float32)
    ps_w = inputs['ps_w'].astype(np.float32)[:, 0]

    # psw rows + partition-broadcast stationary [2, 128], bf16 for 1-pass PE
    from ml_dtypes import bfloat16
    pw = np.zeros((2, HC + 128), np.float32)
    pw[0, 0:HC] = ps_w[0:HC]
    pw[1, 0:HC - 1] = ps_w[HC:N1]
    pw[0, HC:HC + BL] = 1.0
    pw[1, HC + BL:HC + 128] = 1.0
    pw = pw.astype(bfloat16)

    Mx = np.zeros((128, 132), np.float32)
    idx = np.arange(128)
    Mx[idx, idx] = 1.0
    Mx[idx ^ 64, idx] = 1.0
    w2sel = np.zeros((4 * H, 4), np.float32)
    w2sel[0:H, 0] = fc['w2_mu']
    w2sel[H:2 * H, 1] = fc['w2_mu']
    w2sel[2 * H:3 * H, 2] = fc['w2_lv']
    w2sel[3 * H:4 * H, 3] = fc['w2_lv']
    Mx[0:4 * H, 128:132] = w2sel
    mb = Mx.astype(bfloat16)

    consts = np.zeros((128, lay['width'] - K), np.float32)
    for i, v in enumerate(lay['bias_vals']):
        consts[:, lay['bias0'] - K + i] = v
    consts[:, lay['eq01'] - K:lay['eq01'] - K + 2] = [0.0, 1.0]
    consts[:, lay['cA'] - K:lay['cA'] - K + 3] = [1e-4, 1.0 + 1e-4, 1.0]
    consts[:, lay['cB'] - K:lay['cB'] - K + 3] = [1.0 + 1e-4, 1e-4, 1.0]
    posa = np.zeros(4 * H); posc = np.zeros(4 * H)
    posa[0:H] = fc['u_mu'];          posc[0:H] = fc['vb_mu']
    posa[H:2 * H] = fc['u_mu'];      posc[H:2 * H] = fc['vc_mu']
    posa[2 * H:3 * H] = fc['u_lv'];  posc[2 * H:3 * H] = fc['vb_lv']
    posa[3 * H:4 * H] = fc['u_lv'];  posc[3 * H:4 * H] = fc['vc_lv']
    consts[0:4 * H, lay['posa'] - K] = posa
    consts[0:4 * H, lay['posc'] - K] = posc

    in_maps = []
    for i in range(NCORES):
        xs = x[i * BL:(i + 1) * BL]                       # [64, 512]
        xt = np.ascontiguousarray(
            xs.reshape(BL, 2, HC).transpose(1, 0, 2).reshape(128, HC))
        ys = y[i * BL:(i + 1) * BL]
        yv = np.ascontiguousarray(np.vstack([ys, ys]))    # [128, K]
        yc = np.ascontiguousarray(
            np.hstack([yv, consts]).astype(np.float32))   # [128, width]
        in_maps.append({
            'xt': xt, 'yc': yc, 'pw': pw, 'mb': mb,
        })
    return in_maps


def _combine(parts):
    tot = np.zeros((4, K + 1), np.float64)
    for p in parts:
        tot += p.astype(np.float64)
    P0, n0 = tot[0, :K], tot[0, K]
    P1, n1 = tot[1, :K], tot[1, K]
    Q0, r0 = tot[2, :K], tot[2, K]
    Q1, r1 = tot[3, :K], tot[3, K]
    d0 = n0 * (XD - 1)
    d1 = n1 * (XD - 1)
    cmi0 = P0 / d0
    cmi1 = P1 / d1
    dr = 0.5 * ((XD - 1) * cmi0 * (n0 - r0) + Q0) / d0 \
       + 0.5 * ((XD - 1) * cmi1 * (n1 - r1) + Q1) / d1
    cmi_dims = (np.abs(cmi0 + cmi1) / 2.0).astype(np.float32)
    drs = np.abs(dr).astype(np.float32)
    return cmi_dims, drs


def _param_key(inputs, spec):
    import hashlib
    hsh = hashlib.sha256()
    for k in sorted(inputs):
        if k in ('x_samples', 'y_samples'):
            continue
        hsh.update(k.encode())
        hsh.update(np.ascontiguousarray(inputs[k]).tobytes())
    hsh.update(repr(spec).encode())
    return hsh.hexdigest()


def kernel(**inputs):
    from concourse.bass_utils import run_bass_kernel_spmd

    fc = _fold_consts(inputs)
    spec = _specialize(fc, np.asarray(inputs['x_samples']))
    lay = _const_layout(fc, spec)
    key = _param_key(inputs, spec)
    if key not in _prog_cache:
        _prog_cache[key] = _build_program(fc, spec, lay)
    nc = _prog_cache[key]

    in_maps = _host_inputs(inputs, fc, spec, lay)
    res = run_bass_kernel_spmd(nc, in_maps, core_ids=list(range(NCORES)))
    parts = [r['out'] for r in res.results]
    return _combine(parts)
